# revision 28
# baseline (speedup 1.0000x reference)
"""Trainium2 Bass kernel for EncoderWithPositionalAttentionLayer.

Sharding: data-parallel over batch B=8 across 8 NeuronCores (one batch
element per core).  The batch-independent relative-position algebra is
collapsed on the HOST (exact fp32 numpy):

  score[i,j] = q[i].ke[j]/8 + q[i].RW[:,idx] + E1[idx,h] + b0[j] (+consts)
  idx = clip(j-i,-100,100)+100; under the causal mask idx in [0,100].
  Terms constant along a score row (idx=0 tables, bb0/bb1, bkr terms)
  cancel in softmax.  What remains is a banded bias
     Db[i,t] = (q[i]/8).(8*RWD[:,t]) + E1D[t,h],  t = j-i+100 in [1,100]
  with RWD/E1D host-computed delta tables (vs idx=0).

On device, Db goes through a DRAM scratch with read-side skew: rows of
width 360 per (partition, head, itile) hold [127 zeros][100 Db][133 NEG];
one contiguous write, then a read with partition-dependent offset
(stride ROWS-1) yields the j-aligned causal-masked bias window that a
single DVE add applies to each score tile.

Everything on the main path is bf16 (matmul rate is 1 cycle/row, same
as fp32r, at any moving dim; PSUM accumulation stays fp32).  Weights
are host-prepacked partition-major so every weight DMA is 128
contiguous 16KB runs.
"""

import contextlib
import sys

sys.path.insert(0, "/opt/trn_rl_repo")

import numpy as np
import ml_dtypes

import concourse.bass as bass
from concourse import bacc
import concourse.mybir as mybir
import concourse.tile as tile

F32 = mybir.dt.float32
BF16 = mybir.dt.bfloat16
AF = mybir.ActivationFunctionType
ALU = mybir.AluOpType
NPBF = ml_dtypes.bfloat16

B, T, D, H, HID = 8, 512, 512, 8, 2048
DIM = D // H          # 64
L = 100
BW = L                # band width (t = 1..100)
EPS = 1e-3
P = 128
TS = T // P           # 4
DS = D // P           # 4
CS = HID // P         # 16
NEG = -60.0           # exp(-60) ~ 1e-26: exact-enough masking
WROW = 360            # scratch row: [127 zeros][100 Db][133 NEG]
NK = H * TS           # 32 scratch tiles (k = ti*H + h)
ROWS = NK * WROW      # per-partition scratch row block (11520)
WIN = 227             # j-aligned window width read back per tile

# smalls (fp32) column offsets
SM_LN0G, SM_LN0B, SM_LN1G, SM_LN1B = 0, 4, 8, 12
SM_BH0, SM_BH1 = 16, 32
SM_BQ, SM_BKE = 48, 52
SM_ID32 = 56
SM_BKV = SM_ID32 + 128          # 184
SM_BO1 = SM_BKV + 512           # 696
NS = SM_BO1 + 512               # 1208

# smallsb (bf16) column offsets
SB_ID16 = 0
SB_ONES = 128
SB_RWD = 256                    # [128, 4*100]
SB_E1D = SB_RWD + 400           # row 0: 8 heads x 100
SB_MASK = SB_E1D + 800          # row 0: maskbias [T]
SB_WB0 = SB_MASK + 512          # [128, 4*8]
SB_FILL = SB_WB0 + 32           # [128, 360] scratch row fill pattern
NSB = SB_FILL + WROW            # 2388


def build_nc():
    nc = bacc.Bacc()

    dp = nc.declare_dram_parameter
    values = dp("values_b", [P, TS, D], F32, isOutput=False)
    smalls = dp("smalls", [P, NS], F32, isOutput=False)
    smallsb = dp("smallsb_b", [P, NSB], BF16, isOutput=False)
    wh0p = dp("wh0p", [P, DS, HID], BF16, isOutput=False)
    wqp = dp("wqp", [P, CS, D], BF16, isOutput=False)
    wkep = dp("wkep", [P, CS, D], BF16, isOutput=False)
    wkvp = dp("wkvp", [P, CS, D], BF16, isOutput=False)
    wh1p = dp("wh1p", [P, DS, HID], BF16, isOutput=False)
    wo1p = dp("wo1p", [P, CS, D], BF16, isOutput=False)
    out = dp("out_b", [P, TS, D], F32, isOutput=True)

    with tile.TileContext(nc) as tc, contextlib.ExitStack() as ctx:
        persist = ctx.enter_context(tc.tile_pool(name="persist", bufs=1))
        wpool = ctx.enter_context(tc.tile_pool(name="wpool", bufs=3))
        work = ctx.enter_context(tc.tile_pool(name="work", bufs=3))
        psum = ctx.enter_context(tc.tile_pool(name="psum", bufs=3, space="PSUM"))
        psacc = ctx.enter_context(tc.tile_pool(name="psacc", bufs=4, space="PSUM"))
        pwarm = ctx.enter_context(tc.tile_pool(name="pwarm", bufs=1, space="PSUM"))
        dram = ctx.enter_context(tc.tile_pool(name="dram", bufs=1, space="DRAM"))

        # ---------------- input DMAs ----------------------------------------
        # Everything early rides the sync ring: the scalar (ACT) engine
        # spends its first ~10us loading activation tables, which would
        # delay DMAs issued from it.  The scalar ring only carries the
        # mid-kernel scratch round-trip.
        smb = persist.tile([P, NSB], BF16)
        nc.sync.dma_start(smb, smallsb[:, :])
        vals = persist.tile([P, TS, D], F32)
        nc.gpsimd.dma_start(out=vals, in_=values[:, :, :])
        sm = persist.tile([P, NS], F32)
        nc.gpsimd.dma_start(out=sm, in_=smalls[:, :])

        # ---------------- weight DMAs (sync ring, use order) -----------------
        wh0 = wpool.tile([P, DS, HID], BF16, name="wh0", tag="w")
        nc.sync.dma_start(wh0, wh0p[:, :, :])
        wq = wpool.tile([P, CS, D], BF16, name="wq", tag="w")
        nc.sync.dma_start(wq, wqp[:, :, :])
        wke = wpool.tile([P, CS, D], BF16, name="wke", tag="w")
        nc.sync.dma_start(wke, wkep[:, :, :])

        ident32 = sm[:, SM_ID32:SM_ID32 + 128]
        ident16 = smb[:, SB_ID16:SB_ID16 + 128]
        onesb = smb[0:1, SB_ONES:SB_ONES + 128]
        rwdT = smb[:, SB_RWD:SB_RWD + 400].rearrange("p (s t) -> p s t", s=DS)
        wb0_sb = smb[:, SB_WB0:SB_WB0 + 32].rearrange("p (s h) -> p s h", s=DS)
        fill = smb[:, SB_FILL:SB_FILL + WROW]
        eps_sb = persist.tile([P, 1], F32)
        nc.vector.memset(eps_sb, EPS)

        # PE warmers: dependency-free matmuls that keep the HAM clock at
        # 8/8 while real matmul operands are still in flight.
        warm_rhs = smb[:, 0:512]

        def warmers(n):
            for _ in range(n):
                wps = pwarm.tile([P, 512], F32, name="warm", tag="warm")
                nc.tensor.matmul(wps, ident16, warm_rhs, start=True, stop=True)

        warmers(25)

        # scratch fill+band SBUF image: [P, NK, WROW] bf16 (23KB/partition).
        # The fill copies are emitted later (just before the Db section) so
        # they queue behind the LN0 work on DVE instead of ahead of it.
        fb = persist.tile([P, NK, WROW], BF16)

        # --------------------------- LN helper --------------------------------
        def layernorm_to_T(x_tiles, gcol, bcol, lnT_out, name):
            for tt in range(TS):
                xt = x_tiles[:, tt, :]
                stats = work.tile([P, 6], F32, name=f"{name}st{tt}", tag="lnst")
                nc.vector.bn_stats(out=stats, in_=xt)
                mv = work.tile([P, 2], F32, name=f"{name}mv{tt}", tag="lnmv")
                nc.vector.bn_aggr(out=mv, in_=stats)
                rstd = work.tile([P, 1], F32, name=f"{name}rs{tt}", tag="lnrs")
                nc.scalar.activation(out=rstd, in_=mv[:, 1:2], func=AF.Sqrt,
                                     bias=eps_sb, scale=1.0)
                nc.vector.reciprocal(rstd, rstd)
                xn = work.tile([P, D], F32, name=f"{name}xn{tt}", tag="lnxn")
                nc.vector.tensor_scalar(xn, xt, mv[:, 0:1], rstd,
                                        op0=ALU.subtract, op1=ALU.mult)
                for es in range(DS):
                    tp = psum.tile([P, P], F32, name=f"{name}tp", tag="pp")
                    nc.tensor.transpose(tp, xn[:, es * P:(es + 1) * P], ident32)
                    nc.vector.tensor_scalar(
                        lnT_out[:, es, tt * P:(tt + 1) * P], tp,
                        gcol[:, es:es + 1], bcol[:, es:es + 1],
                        op0=ALU.mult, op1=ALU.add)

        # ------------------------- LN0 + block0 ------------------------------
        ln0T = persist.tile([P, DS, T], BF16, name="ln0T", tag="lnT")
        layernorm_to_T(vals, sm[:, SM_LN0G:SM_LN0G + DS],
                       sm[:, SM_LN0B:SM_LN0B + DS], ln0T, "ln0")

        xT = persist.tile([P, CS, T], BF16, name="xT", tag="xT")
        for cs_ in range(CS):
            pp = psacc.tile([P, T], F32, name="h0pp", tag="acc")
            for es in range(DS):
                nc.tensor.matmul(pp, wh0[:, es, cs_ * P:(cs_ + 1) * P],
                                 ln0T[:, es, :],
                                 start=(es == 0), stop=(es == DS - 1))
            nc.vector.tensor_scalar(      # fused bias + relu on DVE
                xT[:, cs_, :], pp, sm[:, SM_BH0 + cs_:SM_BH0 + cs_ + 1], 0.0,
                op0=ALU.add, op1=ALU.max)

        # --------------------------- projections -----------------------------
        def project_T(w_sb, dest, boff, scale):
            """dest [128(d), DS, T] (bf16) = scale*((x @ w).T + b)."""
            accs = [psacc.tile([P, T], F32, name=f"pa{d}", tag="acc")
                    for d in range(DS)]
            for cs_ in range(CS):
                for dsub in range(DS):
                    nc.tensor.matmul(
                        accs[dsub], w_sb[:, cs_, dsub * P:(dsub + 1) * P],
                        xT[:, cs_, :],
                        start=(cs_ == 0), stop=(cs_ == CS - 1))
            for dsub in range(DS):
                nc.vector.tensor_scalar(   # (acc + b) * scale on DVE
                    dest[:, dsub, :], accs[dsub],
                    sm[:, boff + dsub:boff + dsub + 1], scale,
                    op0=ALU.add, op1=ALU.mult)

        qT = persist.tile([P, DS, T], BF16)      # holds q/8 transposed
        project_T(wq, qT, SM_BQ, 0.125)          # bias pre-scaled on host

        # ---------------- Db tiles (interleaved with keT) ---------------------
        # fill margins: row layout [127 zeros][100 Db][133 NEG], two
        # broadcast copies over all NK rows
        nc.vector.tensor_copy(
            fb[:, :, 0:127],
            fill[:, 0:127].rearrange("p (a w) -> p a w", a=1)
                .to_broadcast((P, NK, 127)))
        nc.vector.tensor_copy(
            fb[:, :, WIN:WROW],
            fill[:, WIN:WROW].rearrange("p (a w) -> p a w", a=1)
                .to_broadcast((P, NK, WROW - WIN)))
        hd = lambda h: (h % 2) * DIM

        def db_group(ti, hh):
            """Db for heads 4*hh..4*hh+3 of row-tile ti into one PSUM bank."""
            dbp = psum.tile([P, 4, BW], F32, name="dbp", tag="pp")
            for i4 in range(4):
                h = hh * 4 + i4
                nc.tensor.matmul(
                    dbp[:, i4, :],
                    qT[hd(h):hd(h) + DIM, h // 2, ti * P:(ti + 1) * P],
                    rwdT[hd(h):hd(h) + DIM, h // 2, :],
                    start=True, stop=False)
                nc.tensor.matmul(
                    dbp[:, i4, :], onesb,
                    smb[0:1, SB_E1D + h * BW:SB_E1D + (h + 1) * BW],
                    start=False, stop=True)
            # store exp(Db): the window is applied multiplicatively after
            # the score exp (exp(s+w) = exp(s)*exp(w)); fill is 1.0 / 0.0
            nc.scalar.activation(
                out=fb[:, ti * H + hh * 4:ti * H + hh * 4 + 4, 127:227],
                in_=dbp, func=AF.Exp, bias=0.0, scale=1.0)

        # keT projection with Db groups interleaved so the PE array duty
        # cycle stays high (Db matmuls alone are LDW-dominated)
        keT = persist.tile([P, DS, T], BF16)
        keaccs = [psacc.tile([P, T], F32, name=f"kea{d}", tag="acc")
                  for d in range(DS)]
        for cs_ in range(CS):
            for dsub in range(DS):
                nc.tensor.matmul(
                    keaccs[dsub], wke[:, cs_, dsub * P:(dsub + 1) * P],
                    xT[:, cs_, :],
                    start=(cs_ == 0), stop=(cs_ == CS - 1))
            if cs_ % 2 == 1:
                db_group((cs_ - 1) // 4, ((cs_ - 1) // 2) % 2)
        for dsub in range(DS):
            nc.vector.tensor_scalar(
                keT[:, dsub, :], keaccs[dsub],
                sm[:, SM_BKE + dsub:SM_BKE + dsub + 1], 1.0,
                op0=ALU.add, op1=ALU.mult)

        scr = dram.tile([P * ROWS], BF16, name="scr")
        nc.scalar.dma_start(
            bass.AP(tensor=scr.tensor, offset=scr.offset,
                    ap=[[ROWS, P], [WROW, NK], [1, WROW]]),
            fb)
        # skewed window read: win[p, k, w] = scr[p*ROWS + k*WROW + 127 + w - p]
        wins = []
        for ti in range(TS):
            wr = work.tile([P, H, WIN], BF16, name=f"win{ti}", tag=f"win{ti}",
                           bufs=1)
            nc.scalar.dma_start(
                wr,
                bass.AP(tensor=scr.tensor,
                        offset=scr.offset + ti * H * WROW + 127,
                        ap=[[ROWS - 1, P], [WROW, H], [1, WIN]]))
            wins.append(wr)

        # bias0 (+maskbias) enters the softmax as a per-column factor
        # g[j] = exp(b0[j] + maskbias[j]): fold it into kv, with a 65th
        # column equal to g so attn@kv_aug yields the softmax denominator.
        b0p = psum.tile([H, T], F32, name="b0p", tag="pp")
        for c in range(DS):
            nc.tensor.matmul(b0p, wb0_sb[:, c, :], keT[:, c, :],
                             start=(c == 0), stop=False)
        nc.tensor.matmul(b0p, onesb[:, 0:H], smb[0:1, SB_MASK:SB_MASK + T],
                         start=False, stop=True)
        b0m = work.tile([H, T], F32, name="b0m", tag="b0m", bufs=1)
        nc.vector.tensor_copy(b0m, b0p)
        g = persist.tile([P, TS, H], F32)
        for tt in range(TS):
            gp = psum.tile([P, H], F32, name="gp", tag="pp")
            nc.tensor.matmul(gp, b0m[:, tt * P:(tt + 1) * P],
                             ident32[0:H, 0:H], start=True, stop=True)
            nc.scalar.activation(out=g[:, tt, :], in_=gp, func=AF.Exp,
                                 bias=0.0, scale=1.0)

        wkv = wpool.tile([P, CS, D], BF16, name="wkv", tag="w")
        nc.sync.dma_start(wkv, wkvp[:, :, :])
        kva = persist.tile([P, TS, H, DIM + 1], BF16)
        kvaccs = [psacc.tile([P, D], F32, name=f"kva{t}", tag="acc")
                  for t in range(TS)]
        for cs_ in range(CS):
            for tt in range(TS):
                nc.tensor.matmul(kvaccs[tt],
                                 xT[:, cs_, tt * P:(tt + 1) * P],
                                 wkv[:, cs_, :],
                                 start=(cs_ == 0), stop=(cs_ == CS - 1))
        for tt in range(TS):
            for h in range(H):       # bkv itself is applied via v1 below
                nc.vector.tensor_scalar_mul(
                    kva[:, tt, h, 0:DIM],
                    kvaccs[tt][:, h * DIM:(h + 1) * DIM], g[:, tt, h:h + 1])
            nc.vector.tensor_copy(kva[:, tt, :, DIM], g[:, tt, :])

        # ------------------------------ attention -----------------------------
        # Per ti, three phases so the PE queue never blocks on DVE/ACT:
        # (1) all 8 heads' score matmuls (+DVE bias adds, ACT exp),
        # (2) all probability-tile transposes (DVE copies trail),
        # (3) all attn@v accumulation matmuls (+ACT rz scale-out).
        attn_out = persist.tile([P, TS, D], F32)
        for ti in range(TS):
            nj = (ti + 1) * P
            j0 = ti * P - 99
            warmers(8)
            ats = []
            for h in range(H):
                sp = psacc.tile([P, T], F32, name="sp", tag="acc")
                nc.tensor.matmul(
                    sp[:, 0:nj],
                    qT[hd(h):hd(h) + DIM, h // 2, ti * P:(ti + 1) * P],
                    keT[hd(h):hd(h) + DIM, h // 2, 0:nj],
                    start=True, stop=True)
                if ti == 0:
                    nc.vector.tensor_tensor(sp[:, 0:P], sp[:, 0:P],
                                            wins[0][:, h, 99:WIN], ALU.add)
                else:
                    nc.vector.tensor_tensor(sp[:, j0:j0 + WIN],
                                            sp[:, j0:j0 + WIN],
                                            wins[ti][:, h, :], ALU.add)
                at = work.tile([P, T], BF16, name=f"at{h}", tag=f"at{h}",
                               bufs=1)
                nc.scalar.activation(out=at[:, 0:nj], in_=sp[:, 0:nj],
                                     func=AF.Exp, bias=0.0, scale=1.0)
                ats.append(at)
            atTs = work.tile([P, H, (ti + 1) * P], BF16, name="atTs",
                             tag="atTs", bufs=2)
            for h in range(H):
                tp = psum.tile([P, ti + 1, P], BF16, name="attp", tag="pp")
                for js in range(ti + 1):
                    nc.tensor.transpose(tp[:, js, :],
                                        ats[h][:, js * P:(js + 1) * P],
                                        ident16)
                nc.vector.tensor_copy(
                    atTs[:, h, :].rearrange("p (a b) -> p a b", a=ti + 1), tp)
            for h in range(H):
                op = psacc.tile([P, DIM + 1], F32, name="avp", tag="acc")
                for js in range(ti + 1):
                    nc.tensor.matmul(op, atTs[:, h, js * P:(js + 1) * P],
                                     kva[:, js, h, :],
                                     start=(js == 0), stop=(js == ti))
                rz = work.tile([P, 1], F32, name="rz", tag="rz")
                nc.vector.reciprocal(rz, op[:, DIM:DIM + 1])
                nc.scalar.activation(
                    out=attn_out[:, ti, h * DIM:(h + 1) * DIM],
                    in_=op[:, 0:DIM],
                    func=AF.Identity, bias=0.0, scale=rz)

        # ------------------------ residual + block1 ---------------------------
        wh1 = wpool.tile([P, DS, HID], BF16, name="wh1", tag="w")
        nc.sync.dma_start(wh1, wh1p[:, :, :])
        v1 = persist.tile([P, TS, D], F32)
        warmers(12)
        for tt in range(TS):
            nc.vector.tensor_add(v1[:, tt, :], vals[:, tt, :],
                                 attn_out[:, tt, :])
            nc.vector.tensor_add(v1[:, tt, :], v1[:, tt, :],
                                 sm[:, SM_BKV:SM_BKV + D])
        ln1T = persist.tile([P, DS, T], BF16, name="ln1T", tag="lnT")
        layernorm_to_T(v1, sm[:, SM_LN1G:SM_LN1G + DS],
                       sm[:, SM_LN1B:SM_LN1B + DS], ln1T, "ln1")
        warmers(10)

        x1T = persist.tile([P, CS, T], BF16, name="x1T", tag="xT")
        for cs_ in range(CS):
            pp = psacc.tile([P, T], F32, name="h1pp", tag="acc")
            for es in range(DS):
                nc.tensor.matmul(pp, wh1[:, es, cs_ * P:(cs_ + 1) * P],
                                 ln1T[:, es, :],
                                 start=(es == 0), stop=(es == DS - 1))
            nc.vector.tensor_scalar(
                x1T[:, cs_, :], pp, sm[:, SM_BH1 + cs_:SM_BH1 + cs_ + 1], 0.0,
                op0=ALU.add, op1=ALU.max)

        wo1 = wpool.tile([P, CS, D], BF16, name="wo1", tag="w")
        nc.sync.dma_start(wo1, wo1p[:, :, :])
        for tt in range(TS):       # tt-outer: out DMA per tile starts early
            o1acc = psacc.tile([P, D], F32, name=f"o1a{tt}", tag="acc")
            for cs_ in range(CS):
                nc.tensor.matmul(o1acc,
                                 x1T[:, cs_, tt * P:(tt + 1) * P],
                                 wo1[:, cs_, :],
                                 start=(cs_ == 0), stop=(cs_ == CS - 1))
            fin = work.tile([P, D], F32, name="fin", tag="fin")
            nc.vector.tensor_add(fin, o1acc, v1[:, tt, :])
            nc.vector.tensor_add(fin, fin, sm[:, SM_BO1:SM_BO1 + D])
            nc.sync.dma_start(out[:, tt, :], fin)

    if not nc.is_finalized():
        nc.finalize()
    return nc


def _pcol(v):
    """[D] -> [128, D//128] partition-major columns."""
    return np.ascontiguousarray(v.reshape(-1, P).T)


def _pmajor(w, rows_per_part):
    """[(s p), c] -> [128, s, c]."""
    s = rows_per_part
    return np.ascontiguousarray(
        w.reshape(s, P, w.shape[1]).transpose(1, 0, 2))


def build_in_maps(inputs):
    f32 = lambda x: np.asarray(x, dtype=np.float32)
    bf = lambda x: np.ascontiguousarray(x).astype(NPBF)

    rel101 = f32(inputs["rel_enc"])[:L + 1]                     # [101, D]
    wkr = f32(inputs["wkr"])
    wb1 = f32(inputs["wb1"])
    RW = (rel101 @ wkr).T                                       # [D, 101]
    rwd8 = 8.0 * (RW[:, 1:] - RW[:, 0:1])                       # [D, 100]
    rwdT = rwd8.reshape(DS, P, BW).transpose(1, 0, 2).reshape(P, DS * BW)
    E1 = rel101 @ wkr @ wb1                                     # [101, H]
    e1d = (E1[1:] - E1[0:1]).T                                  # [H, 100]

    smalls = np.zeros((P, NS), np.float32)
    smalls[:, SM_LN0G:SM_LN0G + DS] = _pcol(f32(inputs["ln0_g"]))
    smalls[:, SM_LN0B:SM_LN0B + DS] = _pcol(f32(inputs["ln0_b"]))
    smalls[:, SM_LN1G:SM_LN1G + DS] = _pcol(f32(inputs["ln1_g"]))
    smalls[:, SM_LN1B:SM_LN1B + DS] = _pcol(f32(inputs["ln1_b"]))
    smalls[:, SM_BH0:SM_BH0 + CS] = _pcol(f32(inputs["b_h0"]))
    smalls[:, SM_BH1:SM_BH1 + CS] = _pcol(f32(inputs["b_h1"]))
    smalls[:, SM_BQ:SM_BQ + DS] = _pcol(f32(inputs["bq"]))  # (acc+bq)*0.125
    smalls[:, SM_BKE:SM_BKE + DS] = _pcol(f32(inputs["bke"]))
    smalls[:, SM_ID32:SM_ID32 + P] = np.eye(P, dtype=np.float32)
    smalls[:, SM_BKV:SM_BKV + D] = np.tile(f32(inputs["bkv"]), (P, 1))
    smalls[:, SM_BO1:SM_BO1 + D] = np.tile(f32(inputs["b_o1"]), (P, 1))

    mask = np.asarray(inputs["values_mask"])
    maskbias = np.where(mask, 0.0, NEG).astype(np.float32)      # [B, T]

    smb_base = np.zeros((P, NSB), np.float32)
    smb_base[:, SB_ID16:SB_ID16 + P] = np.eye(P, dtype=np.float32)
    smb_base[0, SB_ONES:SB_ONES + P] = 1.0
    smb_base[:, SB_RWD:SB_RWD + DS * BW] = rwdT
    smb_base[0, SB_E1D:SB_E1D + H * BW] = e1d.reshape(-1)
    smb_base[:, SB_WB0:SB_WB0 + DS * H] = _pmajor(f32(inputs["wb0"]), DS
                                                  ).reshape(P, DS * H)
    fill = np.zeros(WROW, np.float32)
    fill[WIN:] = NEG
    smb_base[:, SB_FILL:SB_FILL + WROW] = fill[None, :]

    shared = {
        "smalls": smalls,
        "wh0p": bf(_pmajor(f32(inputs["w_h0"]), DS)),
        "wqp": bf(_pmajor(f32(inputs["wq"]), CS)),
        "wkep": bf(_pmajor(f32(inputs["wke"]), CS)),
        "wkvp": bf(_pmajor(f32(inputs["wkv"]), CS)),
        "wh1p": bf(_pmajor(f32(inputs["w_h1"]), DS)),
        "wo1p": bf(_pmajor(f32(inputs["w_o1"]), CS)),
    }

    vals = f32(inputs["values"])
    in_maps = []
    for b in range(B):
        m = dict(shared)
        m["values_b"] = np.ascontiguousarray(
            vals[b].reshape(TS, P, D).transpose(1, 0, 2))
        smb = smb_base.copy()
        smb[0, SB_MASK:SB_MASK + T] = maskbias[b]
        m["smallsb_b"] = bf(smb)
        in_maps.append(m)
    return in_maps


_NC_CACHE = None


def kernel(**inputs) -> np.ndarray:
    global _NC_CACHE
    if _NC_CACHE is None:
        _NC_CACHE = build_nc()
    nc = _NC_CACHE

    from concourse.bass_utils import run_bass_kernel_spmd

    in_maps = build_in_maps(inputs)
    res = run_bass_kernel_spmd(nc, in_maps, core_ids=list(range(B)))
    return np.stack(
        [res.results[b]["out_b"].transpose(1, 0, 2).reshape(T, D)
         for b in range(B)], axis=0)


if __name__ == "__main__":
    nc = build_nc()
    print("built ok")


# revision 32
# speedup vs baseline: 1.0588x; 1.0588x over previous
"""Trainium2 Bass kernel for EncoderWithPositionalAttentionLayer.

Sharding: data-parallel over batch B=8 across 8 NeuronCores (one batch
element per core).  The batch-independent relative-position algebra is
collapsed on the HOST (exact fp32 numpy):

  score[i,j] = q[i].ke[j]/8 + q[i].RW[:,idx] + E1[idx,h] + b0[j] (+consts)
  idx = clip(j-i,-100,100)+100; under the causal mask idx in [0,100].
  Terms constant along a score row (idx=0 tables, bb0/bb1, bkr terms)
  cancel in softmax.  What remains is a banded bias
     Db[i,t] = (q[i]/8).(8*RWD[:,t]) + E1D[t,h],  t = j-i+100 in [1,100]
  with RWD/E1D host-computed delta tables (vs idx=0).

On device, Db goes through a DRAM scratch with read-side skew: rows of
width 360 per (partition, head, itile) hold [127 zeros][100 Db][133 NEG];
one contiguous write, then a read with partition-dependent offset
(stride ROWS-1) yields the j-aligned causal-masked bias window that a
single DVE add applies to each score tile.

Everything on the main path is bf16 (matmul rate is 1 cycle/row, same
as fp32r, at any moving dim; PSUM accumulation stays fp32).  Weights
are host-prepacked partition-major so every weight DMA is 128
contiguous 16KB runs.
"""

import contextlib
import sys

sys.path.insert(0, "/opt/trn_rl_repo")

import numpy as np
import ml_dtypes

import concourse.bass as bass
from concourse import bacc
import concourse.mybir as mybir
import concourse.tile as tile

F32 = mybir.dt.float32
BF16 = mybir.dt.bfloat16
AF = mybir.ActivationFunctionType
ALU = mybir.AluOpType
NPBF = ml_dtypes.bfloat16

B, T, D, H, HID = 8, 512, 512, 8, 2048
DIM = D // H          # 64
L = 100
BW = L                # band width (t = 1..100)
EPS = 1e-3
P = 128
TS = T // P           # 4
DS = D // P           # 4
CS = HID // P         # 16
NEG = -60.0           # exp(-60) ~ 1e-26: exact-enough masking
WROW = 360            # scratch row: [127 zeros][100 Db][133 NEG]
NK = H * TS           # 32 scratch tiles (k = ti*H + h)
ROWS = NK * WROW      # per-partition scratch row block (11520)
WIN = 227             # j-aligned window width read back per tile

# smalls (fp32) column offsets
SM_LN0G, SM_LN0B, SM_LN1G, SM_LN1B = 0, 4, 8, 12
SM_BH0, SM_BH1 = 16, 32
SM_BQ, SM_BKE = 48, 52
SM_ID32 = 56
SM_BKV = SM_ID32 + 128          # 184
SM_BO1 = SM_BKV + 512           # 696
NS = SM_BO1 + 512               # 1208

# smallsb (bf16) column offsets
SB_ID16 = 0
SB_ONES = 128
SB_RWD = 256                    # [128, 4*100]
SB_E1D = SB_RWD + 400           # row 0: 8 heads x 100
SB_MASK = SB_E1D + 800          # row 0: maskbias [T]
SB_WB0 = SB_MASK + 512          # [128, 4*8]
SB_FILL = SB_WB0 + 32           # [128, 360] scratch row fill pattern
NSB = SB_FILL + WROW            # 2388


def build_nc():
    nc = bacc.Bacc()

    dp = nc.declare_dram_parameter
    values = dp("values_b", [P, TS, D], F32, isOutput=False)
    smalls = dp("smalls", [P, NS], F32, isOutput=False)
    smallsb = dp("smallsb_b", [P, NSB], BF16, isOutput=False)
    wh0p = dp("wh0p", [P, DS, HID], BF16, isOutput=False)
    wqp = dp("wqp", [P, CS, D], BF16, isOutput=False)
    wkep = dp("wkep", [P, CS, D], BF16, isOutput=False)
    wkvp = dp("wkvp", [P, CS, D], BF16, isOutput=False)
    wh1p = dp("wh1p", [P, DS, HID], BF16, isOutput=False)
    wo1p = dp("wo1p", [P, CS, D], BF16, isOutput=False)
    out = dp("out_b", [P, TS, D], F32, isOutput=True)

    with tile.TileContext(nc) as tc, contextlib.ExitStack() as ctx:
        persist = ctx.enter_context(tc.tile_pool(name="persist", bufs=1))
        wpool = ctx.enter_context(tc.tile_pool(name="wpool", bufs=3))
        work = ctx.enter_context(tc.tile_pool(name="work", bufs=3))
        psum = ctx.enter_context(tc.tile_pool(name="psum", bufs=3, space="PSUM"))
        psacc = ctx.enter_context(tc.tile_pool(name="psacc", bufs=4, space="PSUM"))
        pwarm = ctx.enter_context(tc.tile_pool(name="pwarm", bufs=1, space="PSUM"))
        dram = ctx.enter_context(tc.tile_pool(name="dram", bufs=1, space="DRAM"))

        # ---------------- input DMAs ----------------------------------------
        # Everything early rides the sync ring: the scalar (ACT) engine
        # spends its first ~10us loading activation tables, which would
        # delay DMAs issued from it.  The scalar ring only carries the
        # mid-kernel scratch round-trip.
        smb = persist.tile([P, NSB], BF16)
        nc.sync.dma_start(smb, smallsb[:, :])
        vals = persist.tile([P, TS, D], F32)
        nc.sync.dma_start(vals, values[:, :, :])
        sm = persist.tile([P, NS], F32)
        nc.sync.dma_start(sm, smalls[:, :])

        # ---------------- weight DMAs (sync ring, use order) -----------------
        wh0 = wpool.tile([P, DS, HID], BF16, name="wh0", tag="w")
        nc.sync.dma_start(wh0, wh0p[:, :, :])
        wq = wpool.tile([P, CS, D], BF16, name="wq", tag="w")
        nc.sync.dma_start(wq, wqp[:, :, :])
        wke = wpool.tile([P, CS, D], BF16, name="wke", tag="w")
        nc.sync.dma_start(wke, wkep[:, :, :])

        ident32 = sm[:, SM_ID32:SM_ID32 + 128]
        ident16 = smb[:, SB_ID16:SB_ID16 + 128]
        onesb = smb[0:1, SB_ONES:SB_ONES + 128]
        rwdT = smb[:, SB_RWD:SB_RWD + 400].rearrange("p (s t) -> p s t", s=DS)
        wb0_sb = smb[:, SB_WB0:SB_WB0 + 32].rearrange("p (s h) -> p s h", s=DS)
        fill = smb[:, SB_FILL:SB_FILL + WROW]
        eps_sb = persist.tile([P, 1], F32)
        nc.vector.memset(eps_sb, EPS)

        # PE warmers: dependency-free matmuls that keep the HAM clock at
        # 8/8 while real matmul operands are still in flight.
        warm_rhs = smb[:, 0:512]

        def warmers(n):
            for _ in range(n):
                wps = pwarm.tile([P, 512], F32, name="warm", tag="warm")
                nc.tensor.matmul(wps, ident16, warm_rhs, start=True, stop=True)

        warmers(25)

        # scratch fill+band SBUF image: [P, NK, WROW] bf16 (23KB/partition).
        # The fill copies are emitted later (just before the Db section) so
        # they queue behind the LN0 work on DVE instead of ahead of it.
        fb = persist.tile([P, NK, WROW], BF16)

        # --------------------------- LN helper --------------------------------
        def layernorm_to_T(x_tiles, gcol, bcol, lnT_out, name):
            for tt in range(TS):
                xt = x_tiles[:, tt, :]
                stats = work.tile([P, 6], F32, name=f"{name}st{tt}", tag="lnst")
                nc.vector.bn_stats(out=stats, in_=xt)
                mv = work.tile([P, 2], F32, name=f"{name}mv{tt}", tag="lnmv")
                nc.vector.bn_aggr(out=mv, in_=stats)
                rstd = work.tile([P, 1], F32, name=f"{name}rs{tt}", tag="lnrs")
                nc.scalar.activation(out=rstd, in_=mv[:, 1:2], func=AF.Sqrt,
                                     bias=eps_sb, scale=1.0)
                nc.vector.reciprocal(rstd, rstd)
                xn = work.tile([P, D], F32, name=f"{name}xn{tt}", tag="lnxn")
                nc.vector.tensor_scalar(xn, xt, mv[:, 0:1], rstd,
                                        op0=ALU.subtract, op1=ALU.mult)
                for es in range(DS):
                    tp = psum.tile([P, P], F32, name=f"{name}tp", tag="pp")
                    nc.tensor.transpose(tp, xn[:, es * P:(es + 1) * P], ident32)
                    nc.vector.tensor_scalar(
                        lnT_out[:, es, tt * P:(tt + 1) * P], tp,
                        gcol[:, es:es + 1], bcol[:, es:es + 1],
                        op0=ALU.mult, op1=ALU.add)

        # ------------------------- LN0 + block0 ------------------------------
        ln0T = persist.tile([P, DS, T], BF16, name="ln0T", tag="lnT")
        layernorm_to_T(vals, sm[:, SM_LN0G:SM_LN0G + DS],
                       sm[:, SM_LN0B:SM_LN0B + DS], ln0T, "ln0")

        xT = persist.tile([P, CS, T], BF16, name="xT", tag="xT")
        for cs_ in range(CS):
            pp = psacc.tile([P, T], F32, name="h0pp", tag="acc")
            for es in range(DS):
                nc.tensor.matmul(pp, wh0[:, es, cs_ * P:(cs_ + 1) * P],
                                 ln0T[:, es, :],
                                 start=(es == 0), stop=(es == DS - 1))
            nc.vector.tensor_scalar(      # fused bias + relu on DVE
                xT[:, cs_, :], pp, sm[:, SM_BH0 + cs_:SM_BH0 + cs_ + 1], 0.0,
                op0=ALU.add, op1=ALU.max)

        # --------------------------- projections -----------------------------
        def project_T(w_sb, dest, boff, scale):
            """dest [128(d), DS, T] (bf16) = scale*((x @ w).T + b)."""
            accs = [psacc.tile([P, T], F32, name=f"pa{d}", tag="acc")
                    for d in range(DS)]
            for cs_ in range(CS):
                for dsub in range(DS):
                    nc.tensor.matmul(
                        accs[dsub], w_sb[:, cs_, dsub * P:(dsub + 1) * P],
                        xT[:, cs_, :],
                        start=(cs_ == 0), stop=(cs_ == CS - 1))
            for dsub in range(DS):
                nc.vector.tensor_scalar(   # (acc + b) * scale on DVE
                    dest[:, dsub, :], accs[dsub],
                    sm[:, boff + dsub:boff + dsub + 1], scale,
                    op0=ALU.add, op1=ALU.mult)

        qT = persist.tile([P, DS, T], BF16)      # holds q/8 transposed
        project_T(wq, qT, SM_BQ, 0.125)          # bias pre-scaled on host

        # ---------------- Db tiles (interleaved with keT) ---------------------
        # fill margins: row layout [127 zeros][100 Db][133 NEG], two
        # broadcast copies over all NK rows
        nc.vector.tensor_copy(
            fb[:, :, 0:127],
            fill[:, 0:127].rearrange("p (a w) -> p a w", a=1)
                .to_broadcast((P, NK, 127)))
        nc.vector.tensor_copy(
            fb[:, :, WIN:WROW],
            fill[:, WIN:WROW].rearrange("p (a w) -> p a w", a=1)
                .to_broadcast((P, NK, WROW - WIN)))
        hd = lambda h: (h % 2) * DIM

        def db_group(ti, hh):
            """Db for heads 4*hh..4*hh+3 of row-tile ti into one PSUM bank."""
            dbp = psum.tile([P, 4, BW], F32, name="dbp", tag="pp")
            for i4 in range(4):
                h = hh * 4 + i4
                nc.tensor.matmul(
                    dbp[:, i4, :],
                    qT[hd(h):hd(h) + DIM, h // 2, ti * P:(ti + 1) * P],
                    rwdT[hd(h):hd(h) + DIM, h // 2, :],
                    start=True, stop=False)
                nc.tensor.matmul(
                    dbp[:, i4, :], onesb,
                    smb[0:1, SB_E1D + h * BW:SB_E1D + (h + 1) * BW],
                    start=False, stop=True)
            # store exp(Db): the window is applied multiplicatively after
            # the score exp (exp(s+w) = exp(s)*exp(w)); fill is 1.0 / 0.0
            nc.scalar.activation(
                out=fb[:, ti * H + hh * 4:ti * H + hh * 4 + 4, 127:227],
                in_=dbp, func=AF.Exp, bias=0.0, scale=1.0)

        # keT projection with Db groups interleaved so the PE array duty
        # cycle stays high (Db matmuls alone are LDW-dominated)
        keT = persist.tile([P, DS, T], BF16)
        keaccs = [psacc.tile([P, T], F32, name=f"kea{d}", tag="acc")
                  for d in range(DS)]
        for cs_ in range(CS):
            for dsub in range(DS):
                nc.tensor.matmul(
                    keaccs[dsub], wke[:, cs_, dsub * P:(dsub + 1) * P],
                    xT[:, cs_, :],
                    start=(cs_ == 0), stop=(cs_ == CS - 1))
            if cs_ % 2 == 1:
                db_group((cs_ - 1) // 4, ((cs_ - 1) // 2) % 2)
        for dsub in range(DS):
            nc.vector.tensor_scalar(
                keT[:, dsub, :], keaccs[dsub],
                sm[:, SM_BKE + dsub:SM_BKE + dsub + 1], 1.0,
                op0=ALU.add, op1=ALU.mult)

        scr = dram.tile([P * ROWS], BF16, name="scr")
        nc.scalar.dma_start(
            bass.AP(tensor=scr.tensor, offset=scr.offset,
                    ap=[[ROWS, P], [WROW, NK], [1, WROW]]),
            fb)
        # skewed window read: win[p, k, w] = scr[p*ROWS + k*WROW + 127 + w - p]
        wins = []
        for ti in range(TS):
            wr = work.tile([P, H, WIN], BF16, name=f"win{ti}", tag=f"win{ti}",
                           bufs=1)
            nc.scalar.dma_start(
                wr,
                bass.AP(tensor=scr.tensor,
                        offset=scr.offset + ti * H * WROW + 127,
                        ap=[[ROWS - 1, P], [WROW, H], [1, WIN]]))
            wins.append(wr)

        # bias0 (+maskbias) enters the softmax as a per-column factor
        # g[j] = exp(b0[j] + maskbias[j]): fold it into kv, with a 65th
        # column equal to g so attn@kv_aug yields the softmax denominator.
        b0p = psum.tile([H, T], F32, name="b0p", tag="pp")
        for c in range(DS):
            nc.tensor.matmul(b0p, wb0_sb[:, c, :], keT[:, c, :],
                             start=(c == 0), stop=False)
        nc.tensor.matmul(b0p, onesb[:, 0:H], smb[0:1, SB_MASK:SB_MASK + T],
                         start=False, stop=True)
        b0m = work.tile([H, T], F32, name="b0m", tag="b0m", bufs=1)
        nc.vector.tensor_copy(b0m, b0p)
        g = persist.tile([P, TS, H], F32)
        for tt in range(TS):
            gp = psum.tile([P, H], F32, name="gp", tag="pp")
            nc.tensor.matmul(gp, b0m[:, tt * P:(tt + 1) * P],
                             ident32[0:H, 0:H], start=True, stop=True)
            nc.scalar.activation(out=g[:, tt, :], in_=gp, func=AF.Exp,
                                 bias=0.0, scale=1.0)

        wkv = wpool.tile([P, CS, D], BF16, name="wkv", tag="w")
        nc.sync.dma_start(wkv, wkvp[:, :, :])
        kva = persist.tile([P, TS, H, DIM + 1], BF16)
        kvaccs = [psacc.tile([P, D], F32, name=f"kva{t}", tag="acc")
                  for t in range(TS)]
        for cs_ in range(CS):
            for tt in range(TS):
                nc.tensor.matmul(kvaccs[tt],
                                 xT[:, cs_, tt * P:(tt + 1) * P],
                                 wkv[:, cs_, :],
                                 start=(cs_ == 0), stop=(cs_ == CS - 1))
        for tt in range(TS):
            for h in range(H):       # bkv itself is applied via v1 below
                nc.vector.tensor_scalar_mul(
                    kva[:, tt, h, 0:DIM],
                    kvaccs[tt][:, h * DIM:(h + 1) * DIM], g[:, tt, h:h + 1])
            nc.vector.tensor_copy(kva[:, tt, :, DIM], g[:, tt, :])

        # ------------------------------ attention -----------------------------
        # Per ti, three phases so the PE queue never blocks on DVE/ACT:
        # (1) all 8 heads' score matmuls (+DVE bias adds, ACT exp),
        # (2) all probability-tile transposes (DVE copies trail),
        # (3) all attn@v accumulation matmuls (+ACT rz scale-out).
        attn_out = persist.tile([P, TS, D], F32)
        for ti in range(TS):
            nj = (ti + 1) * P
            j0 = ti * P - 99
            warmers(8 if ti < 2 else 12)
            ats = []
            for h in range(H):
                sp = psacc.tile([P, T], F32, name="sp", tag="acc")
                nc.tensor.matmul(
                    sp[:, 0:nj],
                    qT[hd(h):hd(h) + DIM, h // 2, ti * P:(ti + 1) * P],
                    keT[hd(h):hd(h) + DIM, h // 2, 0:nj],
                    start=True, stop=True)
                at = work.tile([P, T], BF16, name=f"at{h}", tag=f"at{h}",
                               bufs=1)
                nc.scalar.activation(out=at[:, 0:nj], in_=sp[:, 0:nj],
                                     func=AF.Exp, bias=0.0, scale=1.0)
                # banded bias and causal mask enter multiplicatively:
                # window holds exp(Db) in the band, 1.0 below, 0.0 above diag
                if ti == 0:
                    nc.vector.tensor_tensor(at[:, 0:P], at[:, 0:P],
                                            wins[0][:, h, 99:WIN], ALU.mult)
                else:
                    nc.vector.tensor_tensor(at[:, j0:j0 + WIN],
                                            at[:, j0:j0 + WIN],
                                            wins[ti][:, h, :], ALU.mult)
                ats.append(at)
            atTs = work.tile([P, H, (ti + 1) * P], BF16, name="atTs",
                             tag="atTs", bufs=2)
            for h in range(H):
                tp = psum.tile([P, ti + 1, P], BF16, name="attp", tag="pp")
                for js in range(ti + 1):
                    nc.tensor.transpose(tp[:, js, :],
                                        ats[h][:, js * P:(js + 1) * P],
                                        ident16)
                nc.vector.tensor_copy(
                    atTs[:, h, :].rearrange("p (a b) -> p a b", a=ti + 1), tp)
            for h in range(H):
                op = psacc.tile([P, DIM + 1], F32, name="avp", tag="acc")
                for js in range(ti + 1):
                    nc.tensor.matmul(op, atTs[:, h, js * P:(js + 1) * P],
                                     kva[:, js, h, :],
                                     start=(js == 0), stop=(js == ti))
                rz = work.tile([P, 1], F32, name="rz", tag="rz")
                nc.vector.reciprocal(rz, op[:, DIM:DIM + 1])
                nc.scalar.activation(
                    out=attn_out[:, ti, h * DIM:(h + 1) * DIM],
                    in_=op[:, 0:DIM],
                    func=AF.Identity, bias=0.0, scale=rz)

        # ------------------------ residual + block1 ---------------------------
        wh1 = wpool.tile([P, DS, HID], BF16, name="wh1", tag="w")
        nc.sync.dma_start(wh1, wh1p[:, :, :])
        v1 = persist.tile([P, TS, D], F32)
        warmers(12)
        for tt in range(TS):
            nc.vector.tensor_add(v1[:, tt, :], vals[:, tt, :],
                                 attn_out[:, tt, :])
            nc.vector.tensor_add(v1[:, tt, :], v1[:, tt, :],
                                 sm[:, SM_BKV:SM_BKV + D])
        ln1T = persist.tile([P, DS, T], BF16, name="ln1T", tag="lnT")
        layernorm_to_T(v1, sm[:, SM_LN1G:SM_LN1G + DS],
                       sm[:, SM_LN1B:SM_LN1B + DS], ln1T, "ln1")
        warmers(10)

        x1T = persist.tile([P, CS, T], BF16, name="x1T", tag="xT")
        for cs_ in range(CS):
            pp = psacc.tile([P, T], F32, name="h1pp", tag="acc")
            for es in range(DS):
                nc.tensor.matmul(pp, wh1[:, es, cs_ * P:(cs_ + 1) * P],
                                 ln1T[:, es, :],
                                 start=(es == 0), stop=(es == DS - 1))
            nc.vector.tensor_scalar(
                x1T[:, cs_, :], pp, sm[:, SM_BH1 + cs_:SM_BH1 + cs_ + 1], 0.0,
                op0=ALU.add, op1=ALU.max)

        wo1 = wpool.tile([P, CS, D], BF16, name="wo1", tag="w")
        nc.sync.dma_start(wo1, wo1p[:, :, :])
        for tt in range(TS):       # tt-outer: out DMA per tile starts early
            o1acc = psacc.tile([P, D], F32, name=f"o1a{tt}", tag="acc")
            for cs_ in range(CS):
                nc.tensor.matmul(o1acc,
                                 x1T[:, cs_, tt * P:(tt + 1) * P],
                                 wo1[:, cs_, :],
                                 start=(cs_ == 0), stop=(cs_ == CS - 1))
            fin = work.tile([P, D], F32, name="fin", tag="fin")
            nc.vector.tensor_add(fin, o1acc, v1[:, tt, :])
            nc.vector.tensor_add(fin, fin, sm[:, SM_BO1:SM_BO1 + D])
            nc.sync.dma_start(out[:, tt, :], fin)

    if not nc.is_finalized():
        nc.finalize()
    return nc


def _pcol(v):
    """[D] -> [128, D//128] partition-major columns."""
    return np.ascontiguousarray(v.reshape(-1, P).T)


def _pmajor(w, rows_per_part):
    """[(s p), c] -> [128, s, c]."""
    s = rows_per_part
    return np.ascontiguousarray(
        w.reshape(s, P, w.shape[1]).transpose(1, 0, 2))


def build_in_maps(inputs):
    f32 = lambda x: np.asarray(x, dtype=np.float32)
    bf = lambda x: np.ascontiguousarray(x).astype(NPBF)

    rel101 = f32(inputs["rel_enc"])[:L + 1]                     # [101, D]
    wkr = f32(inputs["wkr"])
    wb1 = f32(inputs["wb1"])
    RW = (rel101 @ wkr).T                                       # [D, 101]
    rwd8 = 8.0 * (RW[:, 1:] - RW[:, 0:1])                       # [D, 100]
    rwdT = rwd8.reshape(DS, P, BW).transpose(1, 0, 2).reshape(P, DS * BW)
    E1 = rel101 @ wkr @ wb1                                     # [101, H]
    e1d = (E1[1:] - E1[0:1]).T                                  # [H, 100]

    smalls = np.zeros((P, NS), np.float32)
    smalls[:, SM_LN0G:SM_LN0G + DS] = _pcol(f32(inputs["ln0_g"]))
    smalls[:, SM_LN0B:SM_LN0B + DS] = _pcol(f32(inputs["ln0_b"]))
    smalls[:, SM_LN1G:SM_LN1G + DS] = _pcol(f32(inputs["ln1_g"]))
    smalls[:, SM_LN1B:SM_LN1B + DS] = _pcol(f32(inputs["ln1_b"]))
    smalls[:, SM_BH0:SM_BH0 + CS] = _pcol(f32(inputs["b_h0"]))
    smalls[:, SM_BH1:SM_BH1 + CS] = _pcol(f32(inputs["b_h1"]))
    smalls[:, SM_BQ:SM_BQ + DS] = _pcol(f32(inputs["bq"]))  # (acc+bq)*0.125
    smalls[:, SM_BKE:SM_BKE + DS] = _pcol(f32(inputs["bke"]))
    smalls[:, SM_ID32:SM_ID32 + P] = np.eye(P, dtype=np.float32)
    smalls[:, SM_BKV:SM_BKV + D] = np.tile(f32(inputs["bkv"]), (P, 1))
    smalls[:, SM_BO1:SM_BO1 + D] = np.tile(f32(inputs["b_o1"]), (P, 1))

    mask = np.asarray(inputs["values_mask"])
    maskbias = np.where(mask, 0.0, NEG).astype(np.float32)      # [B, T]

    smb_base = np.zeros((P, NSB), np.float32)
    smb_base[:, SB_ID16:SB_ID16 + P] = np.eye(P, dtype=np.float32)
    smb_base[0, SB_ONES:SB_ONES + P] = 1.0
    smb_base[:, SB_RWD:SB_RWD + DS * BW] = rwdT
    smb_base[0, SB_E1D:SB_E1D + H * BW] = e1d.reshape(-1)
    smb_base[:, SB_WB0:SB_WB0 + DS * H] = _pmajor(f32(inputs["wb0"]), DS
                                                  ).reshape(P, DS * H)
    fill = np.ones(WROW, np.float32)   # multiplicative window: exp(0)=1
    fill[WIN:] = 0.0                   # causal mask: exact zero factor
    smb_base[:, SB_FILL:SB_FILL + WROW] = fill[None, :]

    shared = {
        "smalls": smalls,
        "wh0p": bf(_pmajor(f32(inputs["w_h0"]), DS)),
        "wqp": bf(_pmajor(f32(inputs["wq"]), CS)),
        "wkep": bf(_pmajor(f32(inputs["wke"]), CS)),
        "wkvp": bf(_pmajor(f32(inputs["wkv"]), CS)),
        "wh1p": bf(_pmajor(f32(inputs["w_h1"]), DS)),
        "wo1p": bf(_pmajor(f32(inputs["w_o1"]), CS)),
    }

    vals = f32(inputs["values"])
    in_maps = []
    for b in range(B):
        m = dict(shared)
        m["values_b"] = np.ascontiguousarray(
            vals[b].reshape(TS, P, D).transpose(1, 0, 2))
        smb = smb_base.copy()
        smb[0, SB_MASK:SB_MASK + T] = maskbias[b]
        m["smallsb_b"] = bf(smb)
        in_maps.append(m)
    return in_maps


_NC_CACHE = None


def kernel(**inputs) -> np.ndarray:
    global _NC_CACHE
    if _NC_CACHE is None:
        _NC_CACHE = build_nc()
    nc = _NC_CACHE

    from concourse.bass_utils import run_bass_kernel_spmd

    in_maps = build_in_maps(inputs)
    res = run_bass_kernel_spmd(nc, in_maps, core_ids=list(range(B)))
    return np.stack(
        [res.results[b]["out_b"].transpose(1, 0, 2).reshape(T, D)
         for b in range(B)], axis=0)


if __name__ == "__main__":
    nc = build_nc()
    print("built ok")


# revision 33
# speedup vs baseline: 1.2293x; 1.1610x over previous
"""Trainium2 Bass kernel for EncoderWithPositionalAttentionLayer.

Sharding: data-parallel over batch B=8 across 8 NeuronCores (one batch
element per core).  The batch-independent relative-position algebra is
collapsed on the HOST (exact fp32 numpy):

  score[i,j] = q[i].ke[j]/8 + q[i].RW[:,idx] + E1[idx,h] + b0[j] (+consts)
  idx = clip(j-i,-100,100)+100; under the causal mask idx in [0,100].
  Terms constant along a score row (idx=0 tables, bb0/bb1, bkr terms)
  cancel in softmax.  What remains is a banded bias
     Db[i,t] = (q[i]/8).(8*RWD[:,t]) + E1D[t,h],  t = j-i+100 in [1,100]
  with RWD/E1D host-computed delta tables (vs idx=0).

On device, Db goes through a DRAM scratch with read-side skew: rows of
width 360 per (partition, head, itile) hold [127 zeros][100 Db][133 NEG];
one contiguous write, then a read with partition-dependent offset
(stride ROWS-1) yields the j-aligned causal-masked bias window that a
single DVE add applies to each score tile.

Everything on the main path is bf16 (matmul rate is 1 cycle/row, same
as fp32r, at any moving dim; PSUM accumulation stays fp32).  Weights
are host-prepacked partition-major so every weight DMA is 128
contiguous 16KB runs.
"""

import contextlib
import sys

sys.path.insert(0, "/opt/trn_rl_repo")

import numpy as np
import ml_dtypes

import concourse.bass as bass
from concourse import bacc
import concourse.mybir as mybir
import concourse.tile as tile

F32 = mybir.dt.float32
BF16 = mybir.dt.bfloat16
AF = mybir.ActivationFunctionType
ALU = mybir.AluOpType
NPBF = ml_dtypes.bfloat16

B, T, D, H, HID = 8, 512, 512, 8, 2048
DIM = D // H          # 64
L = 100
BW = L                # band width (t = 1..100)
EPS = 1e-3
P = 128
TS = T // P           # 4
DS = D // P           # 4
CS = HID // P         # 16
NEG = -60.0           # exp(-60) ~ 1e-26: exact-enough masking
WROW = 360            # scratch row: [127 zeros][100 Db][133 NEG]
NK = H * TS           # 32 scratch tiles (k = ti*H + h)
ROWS = NK * WROW      # per-partition scratch row block (11520)
WIN = 227             # j-aligned window width read back per tile

# smalls (fp32) column offsets
SM_LN0G, SM_LN0B, SM_LN1G, SM_LN1B = 0, 4, 8, 12
SM_BH0, SM_BH1 = 16, 32
SM_BQ, SM_BKE = 48, 52
SM_ID32 = 56
SM_BKV = SM_ID32 + 128          # 184
SM_BO1 = SM_BKV + 512           # 696
NS = SM_BO1 + 512               # 1208

# smallsb (bf16) column offsets
SB_ID16 = 0
SB_ONES = 128
SB_RWD = 256                    # [128, 4*100]
SB_E1D = SB_RWD + 400           # row 0: 8 heads x 100
SB_MASK = SB_E1D + 800          # row 0: maskbias [T]
SB_WB0 = SB_MASK + 512          # [128, 4*8]
SB_FILL = SB_WB0 + 32           # [128, 360] scratch row fill pattern
NSB = SB_FILL + WROW            # 2388


def build_nc():
    nc = bacc.Bacc()

    dp = nc.declare_dram_parameter
    values = dp("values_b", [P, TS, D], F32, isOutput=False)
    smalls = dp("smalls", [P, NS], F32, isOutput=False)
    smallsb = dp("smallsb_b", [P, NSB], BF16, isOutput=False)
    wh0p = dp("wh0p", [P, DS, HID], BF16, isOutput=False)
    wqp = dp("wqp", [P, CS, D], BF16, isOutput=False)
    wkep = dp("wkep", [P, CS, D], BF16, isOutput=False)
    wkvp = dp("wkvp", [P, CS, D], BF16, isOutput=False)
    wh1p = dp("wh1p", [P, DS, HID], BF16, isOutput=False)
    wo1p = dp("wo1p", [P, CS, D], BF16, isOutput=False)
    out = dp("out_b", [P, TS, D], F32, isOutput=True)

    with tile.TileContext(nc) as tc, contextlib.ExitStack() as ctx:
        persist = ctx.enter_context(tc.tile_pool(name="persist", bufs=1))
        wpool = ctx.enter_context(tc.tile_pool(name="wpool", bufs=3))
        work = ctx.enter_context(tc.tile_pool(name="work", bufs=3))
        psum = ctx.enter_context(tc.tile_pool(name="psum", bufs=3, space="PSUM"))
        psacc = ctx.enter_context(tc.tile_pool(name="psacc", bufs=4, space="PSUM"))
        pwarm = ctx.enter_context(tc.tile_pool(name="pwarm", bufs=1, space="PSUM"))
        dram = ctx.enter_context(tc.tile_pool(name="dram", bufs=1, space="DRAM"))

        # ---------------- input DMAs ----------------------------------------
        # Everything early rides the sync ring: the scalar (ACT) engine
        # spends its first ~10us loading activation tables, which would
        # delay DMAs issued from it.  The scalar ring only carries the
        # mid-kernel scratch round-trip.
        smb = persist.tile([P, NSB], BF16)
        nc.sync.dma_start(smb, smallsb[:, :])
        vals = persist.tile([P, TS, D], F32)
        nc.sync.dma_start(vals, values[:, :, :])
        sm = persist.tile([P, NS], F32)
        nc.sync.dma_start(sm, smalls[:, :])

        # ---------------- weight DMAs (sync ring, use order) -----------------
        wh0 = wpool.tile([P, DS, HID], BF16, name="wh0", tag="w")
        nc.sync.dma_start(wh0, wh0p[:, :, :])
        wq = wpool.tile([P, CS, D], BF16, name="wq", tag="w")
        nc.sync.dma_start(wq, wqp[:, :, :])
        wke = wpool.tile([P, CS, D], BF16, name="wke", tag="w")
        nc.sync.dma_start(wke, wkep[:, :, :])

        ident32 = sm[:, SM_ID32:SM_ID32 + 128]
        ident16 = smb[:, SB_ID16:SB_ID16 + 128]
        onesb = smb[0:1, SB_ONES:SB_ONES + 128]
        rwdT = smb[:, SB_RWD:SB_RWD + 400].rearrange("p (s t) -> p s t", s=DS)
        wb0_sb = smb[:, SB_WB0:SB_WB0 + 32].rearrange("p (s h) -> p s h", s=DS)
        fill = smb[:, SB_FILL:SB_FILL + WROW]
        eps_sb = persist.tile([P, 1], F32)
        nc.vector.memset(eps_sb, EPS)

        # PE warmers: dependency-free matmuls that keep the HAM clock at
        # 8/8 while real matmul operands are still in flight.
        warm_rhs = smb[:, 0:512]

        def warmers(n):
            for _ in range(n):
                wps = pwarm.tile([P, 512], F32, name="warm", tag="warm")
                nc.tensor.matmul(wps, ident16, warm_rhs, start=True, stop=True)

        warmers(25)

        # scratch fill+band SBUF image: [P, NK, WROW] bf16 (23KB/partition).
        # The fill copies are emitted later (just before the Db section) so
        # they queue behind the LN0 work on DVE instead of ahead of it.
        fb = persist.tile([P, NK, WROW], BF16)

        # --------------------------- LN helper --------------------------------
        def layernorm_to_T(x_tiles, gcol, bcol, lnT_out, name):
            for tt in range(TS):
                xt = x_tiles[:, tt, :]
                stats = work.tile([P, 6], F32, name=f"{name}st{tt}", tag="lnst")
                nc.vector.bn_stats(out=stats, in_=xt)
                mv = work.tile([P, 2], F32, name=f"{name}mv{tt}", tag="lnmv")
                nc.vector.bn_aggr(out=mv, in_=stats)
                rstd = work.tile([P, 1], F32, name=f"{name}rs{tt}", tag="lnrs")
                nc.scalar.activation(out=rstd, in_=mv[:, 1:2], func=AF.Sqrt,
                                     bias=eps_sb, scale=1.0)
                nc.vector.reciprocal(rstd, rstd)
                xn = work.tile([P, D], F32, name=f"{name}xn{tt}", tag="lnxn")
                nc.vector.tensor_scalar(xn, xt, mv[:, 0:1], rstd,
                                        op0=ALU.subtract, op1=ALU.mult)
                for es in range(DS):
                    tp = psum.tile([P, P], F32, name=f"{name}tp", tag="pp")
                    nc.tensor.transpose(tp, xn[:, es * P:(es + 1) * P], ident32)
                    nc.vector.tensor_scalar(
                        lnT_out[:, es, tt * P:(tt + 1) * P], tp,
                        gcol[:, es:es + 1], bcol[:, es:es + 1],
                        op0=ALU.mult, op1=ALU.add)

        # ------------------------- LN0 + block0 ------------------------------
        ln0T = persist.tile([P, DS, T], BF16, name="ln0T", tag="lnT")
        layernorm_to_T(vals, sm[:, SM_LN0G:SM_LN0G + DS],
                       sm[:, SM_LN0B:SM_LN0B + DS], ln0T, "ln0")

        xT = persist.tile([P, CS, T], BF16, name="xT", tag="xT")
        for cs_ in range(CS):
            pp = psacc.tile([P, T], F32, name="h0pp", tag="acc")
            for es in range(DS):
                nc.tensor.matmul(pp, wh0[:, es, cs_ * P:(cs_ + 1) * P],
                                 ln0T[:, es, :],
                                 start=(es == 0), stop=(es == DS - 1))
            nc.vector.tensor_scalar(      # fused bias + relu on DVE
                xT[:, cs_, :], pp, sm[:, SM_BH0 + cs_:SM_BH0 + cs_ + 1], 0.0,
                op0=ALU.add, op1=ALU.max)

        # --------------------------- projections -----------------------------
        def project_T(w_sb, dest, boff, scale):
            """dest [128(d), DS, T] (bf16) = scale*((x @ w).T + b)."""
            accs = [psacc.tile([P, T], F32, name=f"pa{d}", tag="acc")
                    for d in range(DS)]
            for cs_ in range(CS):
                for dsub in range(DS):
                    nc.tensor.matmul(
                        accs[dsub], w_sb[:, cs_, dsub * P:(dsub + 1) * P],
                        xT[:, cs_, :],
                        start=(cs_ == 0), stop=(cs_ == CS - 1))
            for dsub in range(DS):
                nc.vector.tensor_scalar(   # (acc + b) * scale on DVE
                    dest[:, dsub, :], accs[dsub],
                    sm[:, boff + dsub:boff + dsub + 1], scale,
                    op0=ALU.add, op1=ALU.mult)

        qT = persist.tile([P, DS, T], BF16)      # holds q/8 transposed
        project_T(wq, qT, SM_BQ, 0.125)          # bias pre-scaled on host

        # ---------------- Db tiles (interleaved with keT) ---------------------
        # fill margins: row layout [127 zeros][100 Db][133 NEG], two
        # broadcast copies over all NK rows
        nc.vector.tensor_copy(
            fb[:, :, 0:127],
            fill[:, 0:127].rearrange("p (a w) -> p a w", a=1)
                .to_broadcast((P, NK, 127)))
        nc.vector.tensor_copy(
            fb[:, :, WIN:WROW],
            fill[:, WIN:WROW].rearrange("p (a w) -> p a w", a=1)
                .to_broadcast((P, NK, WROW - WIN)))
        hd = lambda h: (h % 2) * DIM

        def db_group(ti, hh):
            """Db for heads 4*hh..4*hh+3 of row-tile ti into one PSUM bank."""
            dbp = psum.tile([P, 4, BW], F32, name="dbp", tag="pp")
            for i4 in range(4):
                h = hh * 4 + i4
                nc.tensor.matmul(
                    dbp[:, i4, :],
                    qT[hd(h):hd(h) + DIM, h // 2, ti * P:(ti + 1) * P],
                    rwdT[hd(h):hd(h) + DIM, h // 2, :],
                    start=True, stop=False)
                nc.tensor.matmul(
                    dbp[:, i4, :], onesb,
                    smb[0:1, SB_E1D + h * BW:SB_E1D + (h + 1) * BW],
                    start=False, stop=True)
            # store exp(Db): the window is applied multiplicatively after
            # the score exp (exp(s+w) = exp(s)*exp(w)); fill is 1.0 / 0.0
            nc.scalar.activation(
                out=fb[:, ti * H + hh * 4:ti * H + hh * 4 + 4, 127:227],
                in_=dbp, func=AF.Exp, bias=0.0, scale=1.0)

        # keT projection with Db groups interleaved so the PE array duty
        # cycle stays high (Db matmuls alone are LDW-dominated)
        keT = persist.tile([P, DS, T], BF16)
        keaccs = [psacc.tile([P, T], F32, name=f"kea{d}", tag="acc")
                  for d in range(DS)]
        for cs_ in range(CS):
            for dsub in range(DS):
                nc.tensor.matmul(
                    keaccs[dsub], wke[:, cs_, dsub * P:(dsub + 1) * P],
                    xT[:, cs_, :],
                    start=(cs_ == 0), stop=(cs_ == CS - 1))
            if cs_ % 2 == 1:
                db_group((cs_ - 1) // 4, ((cs_ - 1) // 2) % 2)
        for dsub in range(DS):
            nc.vector.tensor_scalar(
                keT[:, dsub, :], keaccs[dsub],
                sm[:, SM_BKE + dsub:SM_BKE + dsub + 1], 1.0,
                op0=ALU.add, op1=ALU.mult)

        scr = dram.tile([P * ROWS], BF16, name="scr")
        nc.scalar.dma_start(
            bass.AP(tensor=scr.tensor, offset=scr.offset,
                    ap=[[ROWS, P], [WROW, NK], [1, WROW]]),
            fb)
        # skewed window read: win[p, k, w] = scr[p*ROWS + k*WROW + 127 + w - p]
        wins = []
        for ti in range(TS):
            wr = work.tile([P, H, WIN], BF16, name=f"win{ti}", tag=f"win{ti}",
                           bufs=1)
            nc.scalar.dma_start(
                wr,
                bass.AP(tensor=scr.tensor,
                        offset=scr.offset + ti * H * WROW + 127,
                        ap=[[ROWS - 1, P], [WROW, H], [1, WIN]]))
            wins.append(wr)

        # bias0 (+maskbias) enters the softmax as a per-column factor
        # g[j] = exp(b0[j] + maskbias[j]): fold it into kv, with a 65th
        # column equal to g so attn@kv_aug yields the softmax denominator.
        b0p = psum.tile([H, T], F32, name="b0p", tag="pp")
        for c in range(DS):
            nc.tensor.matmul(b0p, wb0_sb[:, c, :], keT[:, c, :],
                             start=(c == 0), stop=False)
        nc.tensor.matmul(b0p, onesb[:, 0:H], smb[0:1, SB_MASK:SB_MASK + T],
                         start=False, stop=True)
        b0m = work.tile([H, T], F32, name="b0m", tag="b0m", bufs=1)
        nc.vector.tensor_copy(b0m, b0p)
        g = persist.tile([P, TS, H], F32)
        for tt in range(TS):
            gp = psum.tile([P, H], F32, name="gp", tag="pp")
            nc.tensor.matmul(gp, b0m[:, tt * P:(tt + 1) * P],
                             ident32[0:H, 0:H], start=True, stop=True)
            nc.scalar.activation(out=g[:, tt, :], in_=gp, func=AF.Exp,
                                 bias=0.0, scale=1.0)

        wkv = wpool.tile([P, CS, D], BF16, name="wkv", tag="w")
        nc.sync.dma_start(wkv, wkvp[:, :, :])
        kva = persist.tile([P, TS, H, DIM + 1], BF16)
        kvaccs = [psacc.tile([P, D], F32, name=f"kva{t}", tag="acc")
                  for t in range(TS)]
        for cs_ in range(CS):
            for tt in range(TS):
                nc.tensor.matmul(kvaccs[tt],
                                 xT[:, cs_, tt * P:(tt + 1) * P],
                                 wkv[:, cs_, :],
                                 start=(cs_ == 0), stop=(cs_ == CS - 1))
        for tt in range(TS):
            for h in range(H):       # bkv itself is applied via v1 below
                nc.vector.tensor_scalar_mul(
                    kva[:, tt, h, 0:DIM],
                    kvaccs[tt][:, h * DIM:(h + 1) * DIM], g[:, tt, h:h + 1])
            nc.vector.tensor_copy(kva[:, tt, :, DIM], g[:, tt, :])

        # ------------------------------ attention -----------------------------
        # Per ti, three phases so the PE queue never blocks on DVE/ACT:
        # (1) all 8 heads' score matmuls (+DVE bias adds, ACT exp),
        # (2) all probability-tile transposes (DVE copies trail),
        # (3) all attn@v accumulation matmuls (+ACT rz scale-out).
        attn_out = persist.tile([P, TS, D], F32)
        for ti in range(TS):
            nj = (ti + 1) * P
            j0 = ti * P - 99
            warmers(8 if ti < 2 else 12)
            ats = []
            for h in range(H):
                sp = psacc.tile([P, T], F32, name="sp", tag="acc")
                nc.tensor.matmul(
                    sp[:, 0:nj],
                    qT[hd(h):hd(h) + DIM, h // 2, ti * P:(ti + 1) * P],
                    keT[hd(h):hd(h) + DIM, h // 2, 0:nj],
                    start=True, stop=True)
                at = work.tile([P, T], BF16, name=f"at{h}", tag=f"at{h}",
                               bufs=1)
                nc.scalar.activation(out=at[:, 0:nj], in_=sp[:, 0:nj],
                                     func=AF.Exp, bias=0.0, scale=1.0)
                # banded bias and causal mask enter multiplicatively:
                # window holds exp(Db) in the band, 1.0 below, 0.0 above diag
                if ti == 0:
                    nc.vector.tensor_tensor(at[:, 0:P], at[:, 0:P],
                                            wins[0][:, h, 99:WIN], ALU.mult)
                else:
                    nc.vector.tensor_tensor(at[:, j0:j0 + WIN],
                                            at[:, j0:j0 + WIN],
                                            wins[ti][:, h, :], ALU.mult)
                ats.append(at)
            atTs = work.tile([P, H, (ti + 1) * P], BF16, name="atTs",
                             tag="atTs", bufs=2)
            for h in range(H):
                tp = psum.tile([P, ti + 1, P], BF16, name="attp", tag="pp")
                for js in range(ti + 1):
                    nc.tensor.transpose(tp[:, js, :],
                                        ats[h][:, js * P:(js + 1) * P],
                                        ident16)
                nc.vector.tensor_copy(
                    atTs[:, h, :].rearrange("p (a b) -> p a b", a=ti + 1), tp)
            for h in range(H):
                op = psacc.tile([P, DIM + 1], F32, name="avp", tag="acc")
                for js in range(ti + 1):
                    nc.tensor.matmul(op, atTs[:, h, js * P:(js + 1) * P],
                                     kva[:, js, h, :],
                                     start=(js == 0), stop=(js == ti))
                rz = work.tile([P, 1], F32, name="rz", tag="rz")
                nc.vector.reciprocal(rz, op[:, DIM:DIM + 1])
                nc.scalar.activation(
                    out=attn_out[:, ti, h * DIM:(h + 1) * DIM],
                    in_=op[:, 0:DIM],
                    func=AF.Identity, bias=0.0, scale=rz)

        # ------------------------ residual + block1 ---------------------------
        wh1 = wpool.tile([P, DS, HID], BF16, name="wh1", tag="w")
        nc.sync.dma_start(wh1, wh1p[:, :, :])
        v1 = persist.tile([P, TS, D], F32)
        warmers(12)
        for tt in range(TS):
            nc.vector.tensor_add(v1[:, tt, :], vals[:, tt, :],
                                 attn_out[:, tt, :])
            nc.vector.tensor_add(v1[:, tt, :], v1[:, tt, :],
                                 sm[:, SM_BKV:SM_BKV + D])
        ln1T = persist.tile([P, DS, T], BF16, name="ln1T", tag="lnT")
        layernorm_to_T(v1, sm[:, SM_LN1G:SM_LN1G + DS],
                       sm[:, SM_LN1B:SM_LN1B + DS], ln1T, "ln1")
        warmers(10)

        x1T = persist.tile([P, CS, T], BF16, name="x1T", tag="xT")
        for cs_ in range(CS):
            pp = psacc.tile([P, T], F32, name="h1pp", tag="acc")
            for es in range(DS):
                nc.tensor.matmul(pp, wh1[:, es, cs_ * P:(cs_ + 1) * P],
                                 ln1T[:, es, :],
                                 start=(es == 0), stop=(es == DS - 1))
            nc.vector.tensor_scalar(
                x1T[:, cs_, :], pp, sm[:, SM_BH1 + cs_:SM_BH1 + cs_ + 1], 0.0,
                op0=ALU.add, op1=ALU.max)

        wo1 = wpool.tile([P, CS, D], BF16, name="wo1", tag="w")
        nc.sync.dma_start(wo1, wo1p[:, :, :])
        o1accs = [psacc.tile([P, D], F32, name=f"o1a{t}", tag="acc")
                  for t in range(TS)]
        for cs_ in range(CS):      # cs_-outer: o1 starts on x1T chunk 0,
            for tt in range(TS):   # overlapping the rest of the h1 phase
                nc.tensor.matmul(o1accs[tt],
                                 x1T[:, cs_, tt * P:(tt + 1) * P],
                                 wo1[:, cs_, :],
                                 start=(cs_ == 0), stop=(cs_ == CS - 1))
        for tt in range(TS):
            fin = work.tile([P, D], F32, name="fin", tag="fin")
            nc.vector.tensor_add(fin, o1accs[tt], v1[:, tt, :])
            nc.vector.tensor_add(fin, fin, sm[:, SM_BO1:SM_BO1 + D])
            nc.sync.dma_start(out[:, tt, :], fin)

    if not nc.is_finalized():
        nc.finalize()
    return nc


def _pcol(v):
    """[D] -> [128, D//128] partition-major columns."""
    return np.ascontiguousarray(v.reshape(-1, P).T)


def _pmajor(w, rows_per_part):
    """[(s p), c] -> [128, s, c]."""
    s = rows_per_part
    return np.ascontiguousarray(
        w.reshape(s, P, w.shape[1]).transpose(1, 0, 2))


def build_in_maps(inputs):
    f32 = lambda x: np.asarray(x, dtype=np.float32)
    bf = lambda x: np.ascontiguousarray(x).astype(NPBF)

    rel101 = f32(inputs["rel_enc"])[:L + 1]                     # [101, D]
    wkr = f32(inputs["wkr"])
    wb1 = f32(inputs["wb1"])
    RW = (rel101 @ wkr).T                                       # [D, 101]
    rwd8 = 8.0 * (RW[:, 1:] - RW[:, 0:1])                       # [D, 100]
    rwdT = rwd8.reshape(DS, P, BW).transpose(1, 0, 2).reshape(P, DS * BW)
    E1 = rel101 @ wkr @ wb1                                     # [101, H]
    e1d = (E1[1:] - E1[0:1]).T                                  # [H, 100]

    smalls = np.zeros((P, NS), np.float32)
    smalls[:, SM_LN0G:SM_LN0G + DS] = _pcol(f32(inputs["ln0_g"]))
    smalls[:, SM_LN0B:SM_LN0B + DS] = _pcol(f32(inputs["ln0_b"]))
    smalls[:, SM_LN1G:SM_LN1G + DS] = _pcol(f32(inputs["ln1_g"]))
    smalls[:, SM_LN1B:SM_LN1B + DS] = _pcol(f32(inputs["ln1_b"]))
    smalls[:, SM_BH0:SM_BH0 + CS] = _pcol(f32(inputs["b_h0"]))
    smalls[:, SM_BH1:SM_BH1 + CS] = _pcol(f32(inputs["b_h1"]))
    smalls[:, SM_BQ:SM_BQ + DS] = _pcol(f32(inputs["bq"]))  # (acc+bq)*0.125
    smalls[:, SM_BKE:SM_BKE + DS] = _pcol(f32(inputs["bke"]))
    smalls[:, SM_ID32:SM_ID32 + P] = np.eye(P, dtype=np.float32)
    smalls[:, SM_BKV:SM_BKV + D] = np.tile(f32(inputs["bkv"]), (P, 1))
    smalls[:, SM_BO1:SM_BO1 + D] = np.tile(f32(inputs["b_o1"]), (P, 1))

    mask = np.asarray(inputs["values_mask"])
    maskbias = np.where(mask, 0.0, NEG).astype(np.float32)      # [B, T]

    smb_base = np.zeros((P, NSB), np.float32)
    smb_base[:, SB_ID16:SB_ID16 + P] = np.eye(P, dtype=np.float32)
    smb_base[0, SB_ONES:SB_ONES + P] = 1.0
    smb_base[:, SB_RWD:SB_RWD + DS * BW] = rwdT
    smb_base[0, SB_E1D:SB_E1D + H * BW] = e1d.reshape(-1)
    smb_base[:, SB_WB0:SB_WB0 + DS * H] = _pmajor(f32(inputs["wb0"]), DS
                                                  ).reshape(P, DS * H)
    fill = np.ones(WROW, np.float32)   # multiplicative window: exp(0)=1
    fill[WIN:] = 0.0                   # causal mask: exact zero factor
    smb_base[:, SB_FILL:SB_FILL + WROW] = fill[None, :]

    shared = {
        "smalls": smalls,
        "wh0p": bf(_pmajor(f32(inputs["w_h0"]), DS)),
        "wqp": bf(_pmajor(f32(inputs["wq"]), CS)),
        "wkep": bf(_pmajor(f32(inputs["wke"]), CS)),
        "wkvp": bf(_pmajor(f32(inputs["wkv"]), CS)),
        "wh1p": bf(_pmajor(f32(inputs["w_h1"]), DS)),
        "wo1p": bf(_pmajor(f32(inputs["w_o1"]), CS)),
    }

    vals = f32(inputs["values"])
    in_maps = []
    for b in range(B):
        m = dict(shared)
        m["values_b"] = np.ascontiguousarray(
            vals[b].reshape(TS, P, D).transpose(1, 0, 2))
        smb = smb_base.copy()
        smb[0, SB_MASK:SB_MASK + T] = maskbias[b]
        m["smallsb_b"] = bf(smb)
        in_maps.append(m)
    return in_maps


_NC_CACHE = None


def kernel(**inputs) -> np.ndarray:
    global _NC_CACHE
    if _NC_CACHE is None:
        _NC_CACHE = build_nc()
    nc = _NC_CACHE

    from concourse.bass_utils import run_bass_kernel_spmd

    in_maps = build_in_maps(inputs)
    res = run_bass_kernel_spmd(nc, in_maps, core_ids=list(range(B)))
    return np.stack(
        [res.results[b]["out_b"].transpose(1, 0, 2).reshape(T, D)
         for b in range(B)], axis=0)


if __name__ == "__main__":
    nc = build_nc()
    print("built ok")


# revision 34
# speedup vs baseline: 1.2302x; 1.0007x over previous
"""Trainium2 Bass kernel for EncoderWithPositionalAttentionLayer.

Sharding: data-parallel over batch B=8 across 8 NeuronCores (one batch
element per core).  The batch-independent relative-position algebra is
collapsed on the HOST (exact fp32 numpy):

  score[i,j] = q[i].ke[j]/8 + q[i].RW[:,idx] + E1[idx,h] + b0[j] (+consts)
  idx = clip(j-i,-100,100)+100; under the causal mask idx in [0,100].
  Terms constant along a score row (idx=0 tables, bb0/bb1, bkr terms)
  cancel in softmax.  What remains is a banded bias
     Db[i,t] = (q[i]/8).(8*RWD[:,t]) + E1D[t,h],  t = j-i+100 in [1,100]
  with RWD/E1D host-computed delta tables (vs idx=0).

On device, exp(Db) goes through a DRAM scratch with read-side skew:
rows of width 360 per (partition, head, itile) hold
[127 ones][100 exp(Db)][133 zeros]; one contiguous write, then a read
with partition-dependent offset (stride ROWS-1) yields a j-aligned
window that multiplies each probability tile after the score exp
(exp(s+w) = exp(s)*exp(w)), applying band bias AND causal mask at once.
bias0 (+mask bias) enters the same way as a per-column factor
g[j] = exp(b0[j]) folded into kv, whose appended 65th column (= g)
makes attn @ kv_aug produce the softmax denominator for free.

Everything on the main path is bf16 (matmul rate is 1 cycle/row, same
as fp32r, at any moving dim; PSUM accumulation stays fp32).  Weights
are host-prepacked partition-major so every weight DMA is 128
contiguous 16KB runs.
"""

import contextlib
import sys

sys.path.insert(0, "/opt/trn_rl_repo")

import numpy as np
import ml_dtypes

import concourse.bass as bass
from concourse import bacc
import concourse.mybir as mybir
import concourse.tile as tile

F32 = mybir.dt.float32
BF16 = mybir.dt.bfloat16
AF = mybir.ActivationFunctionType
ALU = mybir.AluOpType
NPBF = ml_dtypes.bfloat16

B, T, D, H, HID = 8, 512, 512, 8, 2048
DIM = D // H          # 64
L = 100
BW = L                # band width (t = 1..100)
EPS = 1e-3
P = 128
TS = T // P           # 4
DS = D // P           # 4
CS = HID // P         # 16
NEG = -60.0           # exp(-60) ~ 1e-26: exact-enough masking
WROW = 360            # scratch row: [127 zeros][100 Db][133 NEG]
NK = H * TS           # 32 scratch tiles (k = ti*H + h)
ROWS = NK * WROW      # per-partition scratch row block (11520)
WIN = 227             # j-aligned window width read back per tile

# smalls (fp32) column offsets
SM_LN0G, SM_LN0B, SM_LN1G, SM_LN1B = 0, 4, 8, 12
SM_BH0, SM_BH1 = 16, 32
SM_BQ, SM_BKE = 48, 52
SM_ID32 = 56
SM_BKV = SM_ID32 + 128          # 184
SM_BO1 = SM_BKV + 512           # 696
NS = SM_BO1 + 512               # 1208

# smallsb (bf16) column offsets
SB_ID16 = 0
SB_ONES = 128
SB_RWD = 256                    # [128, 4*100]
SB_E1D = SB_RWD + 400           # row 0: 8 heads x 100
SB_MASK = SB_E1D + 800          # row 0: maskbias [T]
SB_WB0 = SB_MASK + 512          # [128, 4*8]
SB_FILL = SB_WB0 + 32           # [128, 360] scratch row fill pattern
NSB = SB_FILL + WROW            # 2388


def build_nc():
    nc = bacc.Bacc()

    dp = nc.declare_dram_parameter
    values = dp("values_b", [P, TS, D], F32, isOutput=False)
    smalls = dp("smalls", [P, NS], F32, isOutput=False)
    smallsb = dp("smallsb_b", [P, NSB], BF16, isOutput=False)
    wh0p = dp("wh0p", [P, DS, HID], BF16, isOutput=False)
    wqp = dp("wqp", [P, CS, D], BF16, isOutput=False)
    wkep = dp("wkep", [P, CS, D], BF16, isOutput=False)
    wkvp = dp("wkvp", [P, CS, D], BF16, isOutput=False)
    wh1p = dp("wh1p", [P, DS, HID], BF16, isOutput=False)
    wo1p = dp("wo1p", [P, CS, D], BF16, isOutput=False)
    out = dp("out_b", [P, TS, D], F32, isOutput=True)

    with tile.TileContext(nc) as tc, contextlib.ExitStack() as ctx:
        persist = ctx.enter_context(tc.tile_pool(name="persist", bufs=1))
        wpool = ctx.enter_context(tc.tile_pool(name="wpool", bufs=3))
        work = ctx.enter_context(tc.tile_pool(name="work", bufs=3))
        psum = ctx.enter_context(tc.tile_pool(name="psum", bufs=3, space="PSUM"))
        psacc = ctx.enter_context(tc.tile_pool(name="psacc", bufs=4, space="PSUM"))
        pwarm = ctx.enter_context(tc.tile_pool(name="pwarm", bufs=1, space="PSUM"))
        dram = ctx.enter_context(tc.tile_pool(name="dram", bufs=1, space="DRAM"))

        # ---------------- input DMAs ----------------------------------------
        # Everything early rides the sync ring: the scalar (ACT) engine
        # spends its first ~10us loading activation tables, which would
        # delay DMAs issued from it.  The scalar ring only carries the
        # mid-kernel scratch round-trip.
        smb = persist.tile([P, NSB], BF16)
        nc.sync.dma_start(smb, smallsb[:, :])
        vals = persist.tile([P, TS, D], F32)
        nc.sync.dma_start(vals, values[:, :, :])
        sm = persist.tile([P, NS], F32)
        nc.sync.dma_start(sm, smalls[:, :])

        # ---------------- weight DMAs (sync ring, use order) -----------------
        wh0 = wpool.tile([P, DS, HID], BF16, name="wh0", tag="w")
        nc.sync.dma_start(wh0, wh0p[:, :, :])
        wq = wpool.tile([P, CS, D], BF16, name="wq", tag="w")
        nc.sync.dma_start(wq, wqp[:, :, :])
        wke = wpool.tile([P, CS, D], BF16, name="wke", tag="w")
        nc.sync.dma_start(wke, wkep[:, :, :])

        ident32 = sm[:, SM_ID32:SM_ID32 + 128]
        ident16 = smb[:, SB_ID16:SB_ID16 + 128]
        onesb = smb[0:1, SB_ONES:SB_ONES + 128]
        rwdT = smb[:, SB_RWD:SB_RWD + 400].rearrange("p (s t) -> p s t", s=DS)
        wb0_sb = smb[:, SB_WB0:SB_WB0 + 32].rearrange("p (s h) -> p s h", s=DS)
        fill = smb[:, SB_FILL:SB_FILL + WROW]
        eps_sb = persist.tile([P, 1], F32)
        nc.vector.memset(eps_sb, EPS)

        # PE warmers: dependency-free matmuls that keep the HAM clock at
        # 8/8 while real matmul operands are still in flight.
        warm_rhs = smb[:, 0:512]

        def warmers(n):
            for _ in range(n):
                wps = pwarm.tile([P, 512], F32, name="warm", tag="warm")
                nc.tensor.matmul(wps, ident16, warm_rhs, start=True, stop=True)

        warmers(25)

        # scratch fill+band SBUF image: [P, NK, WROW] bf16 (23KB/partition).
        # The fill copies are emitted later (just before the Db section) so
        # they queue behind the LN0 work on DVE instead of ahead of it.
        fb = persist.tile([P, NK, WROW], BF16)

        # --------------------------- LN helper --------------------------------
        def layernorm_to_T(x_tiles, gcol, bcol, lnT_out, name):
            for tt in range(TS):
                xt = x_tiles[:, tt, :]
                stats = work.tile([P, 6], F32, name=f"{name}st{tt}", tag="lnst")
                nc.vector.bn_stats(out=stats, in_=xt)
                mv = work.tile([P, 2], F32, name=f"{name}mv{tt}", tag="lnmv")
                nc.vector.bn_aggr(out=mv, in_=stats)
                rstd = work.tile([P, 1], F32, name=f"{name}rs{tt}", tag="lnrs")
                nc.scalar.activation(out=rstd, in_=mv[:, 1:2], func=AF.Sqrt,
                                     bias=eps_sb, scale=1.0)
                nc.vector.reciprocal(rstd, rstd)
                xn = work.tile([P, D], F32, name=f"{name}xn{tt}", tag="lnxn")
                nc.vector.tensor_scalar(xn, xt, mv[:, 0:1], rstd,
                                        op0=ALU.subtract, op1=ALU.mult)
                for es in range(DS):
                    tp = psum.tile([P, P], F32, name=f"{name}tp", tag="pp")
                    nc.tensor.transpose(tp, xn[:, es * P:(es + 1) * P], ident32)
                    nc.vector.tensor_scalar(
                        lnT_out[:, es, tt * P:(tt + 1) * P], tp,
                        gcol[:, es:es + 1], bcol[:, es:es + 1],
                        op0=ALU.mult, op1=ALU.add)

        # ------------------------- LN0 + block0 ------------------------------
        ln0T = persist.tile([P, DS, T], BF16, name="ln0T", tag="lnT")
        layernorm_to_T(vals, sm[:, SM_LN0G:SM_LN0G + DS],
                       sm[:, SM_LN0B:SM_LN0B + DS], ln0T, "ln0")

        xT = persist.tile([P, CS, T], BF16, name="xT", tag="xT")
        for cs_ in range(CS):
            pp = psacc.tile([P, T], F32, name="h0pp", tag="acc")
            for es in range(DS):
                nc.tensor.matmul(pp, wh0[:, es, cs_ * P:(cs_ + 1) * P],
                                 ln0T[:, es, :],
                                 start=(es == 0), stop=(es == DS - 1))
            nc.vector.tensor_scalar(      # fused bias + relu on DVE
                xT[:, cs_, :], pp, sm[:, SM_BH0 + cs_:SM_BH0 + cs_ + 1], 0.0,
                op0=ALU.add, op1=ALU.max)

        # --------------------------- projections -----------------------------
        def project_T(w_sb, dest, boff, scale):
            """dest [128(d), DS, T] (bf16) = scale*((x @ w).T + b)."""
            accs = [psacc.tile([P, T], F32, name=f"pa{d}", tag="acc")
                    for d in range(DS)]
            for cs_ in range(CS):
                for dsub in range(DS):
                    nc.tensor.matmul(
                        accs[dsub], w_sb[:, cs_, dsub * P:(dsub + 1) * P],
                        xT[:, cs_, :],
                        start=(cs_ == 0), stop=(cs_ == CS - 1))
            for dsub in range(DS):
                nc.vector.tensor_scalar(   # (acc + b) * scale on DVE
                    dest[:, dsub, :], accs[dsub],
                    sm[:, boff + dsub:boff + dsub + 1], scale,
                    op0=ALU.add, op1=ALU.mult)

        qT = persist.tile([P, DS, T], BF16)      # holds q/8 transposed
        project_T(wq, qT, SM_BQ, 0.125)          # bias pre-scaled on host

        # ---------------- Db tiles (interleaved with keT) ---------------------
        # fill margins: row layout [127 zeros][100 Db][133 NEG], two
        # broadcast copies over all NK rows
        nc.vector.tensor_copy(
            fb[:, :, 0:127],
            fill[:, 0:127].rearrange("p (a w) -> p a w", a=1)
                .to_broadcast((P, NK, 127)))
        nc.vector.tensor_copy(
            fb[:, :, WIN:WROW],
            fill[:, WIN:WROW].rearrange("p (a w) -> p a w", a=1)
                .to_broadcast((P, NK, WROW - WIN)))
        hd = lambda h: (h % 2) * DIM

        def db_group(ti, hh):
            """Db for heads 4*hh..4*hh+3 of row-tile ti into one PSUM bank."""
            dbp = psum.tile([P, 4, BW], F32, name="dbp", tag="pp")
            for i4 in range(4):
                h = hh * 4 + i4
                nc.tensor.matmul(
                    dbp[:, i4, :],
                    qT[hd(h):hd(h) + DIM, h // 2, ti * P:(ti + 1) * P],
                    rwdT[hd(h):hd(h) + DIM, h // 2, :],
                    start=True, stop=False)
                nc.tensor.matmul(
                    dbp[:, i4, :], onesb,
                    smb[0:1, SB_E1D + h * BW:SB_E1D + (h + 1) * BW],
                    start=False, stop=True)
            # store exp(Db): the window is applied multiplicatively after
            # the score exp (exp(s+w) = exp(s)*exp(w)); fill is 1.0 / 0.0
            nc.scalar.activation(
                out=fb[:, ti * H + hh * 4:ti * H + hh * 4 + 4, 127:227],
                in_=dbp, func=AF.Exp, bias=0.0, scale=1.0)

        # keT projection with Db groups interleaved so the PE array duty
        # cycle stays high (Db matmuls alone are LDW-dominated)
        keT = persist.tile([P, DS, T], BF16)
        keaccs = [psacc.tile([P, T], F32, name=f"kea{d}", tag="acc")
                  for d in range(DS)]
        for cs_ in range(CS):
            for dsub in range(DS):
                nc.tensor.matmul(
                    keaccs[dsub], wke[:, cs_, dsub * P:(dsub + 1) * P],
                    xT[:, cs_, :],
                    start=(cs_ == 0), stop=(cs_ == CS - 1))
            if cs_ % 2 == 1:
                db_group((cs_ - 1) // 4, ((cs_ - 1) // 2) % 2)
        for dsub in range(DS):
            nc.vector.tensor_scalar(
                keT[:, dsub, :], keaccs[dsub],
                sm[:, SM_BKE + dsub:SM_BKE + dsub + 1], 1.0,
                op0=ALU.add, op1=ALU.mult)

        scr = dram.tile([P * ROWS], BF16, name="scr")
        nc.scalar.dma_start(
            bass.AP(tensor=scr.tensor, offset=scr.offset,
                    ap=[[ROWS, P], [WROW, NK], [1, WROW]]),
            fb)
        # skewed window read: win[p, k, w] = scr[p*ROWS + k*WROW + 127 + w - p]
        wins = []
        for ti in range(TS):
            wr = work.tile([P, H, WIN], BF16, name=f"win{ti}", tag=f"win{ti}",
                           bufs=1)
            nc.scalar.dma_start(
                wr,
                bass.AP(tensor=scr.tensor,
                        offset=scr.offset + ti * H * WROW + 127,
                        ap=[[ROWS - 1, P], [WROW, H], [1, WIN]]))
            wins.append(wr)

        # bias0 (+maskbias) enters the softmax as a per-column factor
        # g[j] = exp(b0[j] + maskbias[j]): fold it into kv, with a 65th
        # column equal to g so attn@kv_aug yields the softmax denominator.
        b0p = psum.tile([H, T], F32, name="b0p", tag="pp")
        for c in range(DS):
            nc.tensor.matmul(b0p, wb0_sb[:, c, :], keT[:, c, :],
                             start=(c == 0), stop=False)
        nc.tensor.matmul(b0p, onesb[:, 0:H], smb[0:1, SB_MASK:SB_MASK + T],
                         start=False, stop=True)
        b0m = work.tile([H, T], F32, name="b0m", tag="b0m", bufs=1)
        nc.vector.tensor_copy(b0m, b0p)
        g = persist.tile([P, TS, H], F32)
        for tt in range(TS):
            gp = psum.tile([P, H], F32, name="gp", tag="pp")
            nc.tensor.matmul(gp, b0m[:, tt * P:(tt + 1) * P],
                             ident32[0:H, 0:H], start=True, stop=True)
            nc.scalar.activation(out=g[:, tt, :], in_=gp, func=AF.Exp,
                                 bias=0.0, scale=1.0)

        wkv = wpool.tile([P, CS, D], BF16, name="wkv", tag="w")
        nc.sync.dma_start(wkv, wkvp[:, :, :])
        kva = persist.tile([P, TS, H, DIM + 1], BF16)
        kvaccs = [psacc.tile([P, D], F32, name=f"kva{t}", tag="acc")
                  for t in range(TS)]
        for cs_ in range(CS):
            for tt in range(TS):
                nc.tensor.matmul(kvaccs[tt],
                                 xT[:, cs_, tt * P:(tt + 1) * P],
                                 wkv[:, cs_, :],
                                 start=(cs_ == 0), stop=(cs_ == CS - 1))
        for tt in range(TS):
            for h in range(H):       # bkv itself is applied via v1 below
                nc.vector.tensor_scalar_mul(
                    kva[:, tt, h, 0:DIM],
                    kvaccs[tt][:, h * DIM:(h + 1) * DIM], g[:, tt, h:h + 1])
            nc.vector.tensor_copy(kva[:, tt, :, DIM], g[:, tt, :])

        # ------------------------------ attention -----------------------------
        # Per ti, three phases so the PE queue never blocks on DVE/ACT:
        # (1) all 8 heads' score matmuls (+DVE bias adds, ACT exp),
        # (2) all probability-tile transposes (DVE copies trail),
        # (3) all attn@v accumulation matmuls (+ACT rz scale-out).
        attn_out = persist.tile([P, TS, D], F32)
        for ti in range(TS):
            nj = (ti + 1) * P
            j0 = ti * P - 99
            warmers(8 if ti < 2 else 12)
            ats = []
            for h in range(H):
                sp = psacc.tile([P, T], F32, name="sp", tag="acc")
                nc.tensor.matmul(
                    sp[:, 0:nj],
                    qT[hd(h):hd(h) + DIM, h // 2, ti * P:(ti + 1) * P],
                    keT[hd(h):hd(h) + DIM, h // 2, 0:nj],
                    start=True, stop=True)
                at = work.tile([P, T], BF16, name=f"at{h}", tag=f"at{h}",
                               bufs=1)
                nc.scalar.activation(out=at[:, 0:nj], in_=sp[:, 0:nj],
                                     func=AF.Exp, bias=0.0, scale=1.0)
                # banded bias and causal mask enter multiplicatively:
                # window holds exp(Db) in the band, 1.0 below, 0.0 above diag
                if ti == 0:
                    nc.vector.tensor_tensor(at[:, 0:P], at[:, 0:P],
                                            wins[0][:, h, 99:WIN], ALU.mult)
                else:
                    nc.vector.tensor_tensor(at[:, j0:j0 + WIN],
                                            at[:, j0:j0 + WIN],
                                            wins[ti][:, h, :], ALU.mult)
                ats.append(at)
            atTs = work.tile([P, H, (ti + 1) * P], BF16, name="atTs",
                             tag="atTs", bufs=2)
            for h in range(H):
                tp = psum.tile([P, ti + 1, P], BF16, name="attp", tag="pp")
                for js in range(ti + 1):
                    nc.tensor.transpose(tp[:, js, :],
                                        ats[h][:, js * P:(js + 1) * P],
                                        ident16)
                nc.vector.tensor_copy(
                    atTs[:, h, :].rearrange("p (a b) -> p a b", a=ti + 1), tp)
            for h in range(H):
                op = psacc.tile([P, DIM + 1], F32, name="avp", tag="acc")
                for js in range(ti + 1):
                    nc.tensor.matmul(op, atTs[:, h, js * P:(js + 1) * P],
                                     kva[:, js, h, :],
                                     start=(js == 0), stop=(js == ti))
                rz = work.tile([P, 1], F32, name="rz", tag="rz")
                nc.vector.reciprocal(rz, op[:, DIM:DIM + 1])
                nc.scalar.activation(
                    out=attn_out[:, ti, h * DIM:(h + 1) * DIM],
                    in_=op[:, 0:DIM],
                    func=AF.Identity, bias=0.0, scale=rz)

        # ------------------------ residual + block1 ---------------------------
        wh1 = wpool.tile([P, DS, HID], BF16, name="wh1", tag="w")
        nc.sync.dma_start(wh1, wh1p[:, :, :])
        v1 = persist.tile([P, TS, D], F32)
        warmers(12)
        for tt in range(TS):
            nc.vector.tensor_add(v1[:, tt, :], vals[:, tt, :],
                                 attn_out[:, tt, :])
            nc.vector.tensor_add(v1[:, tt, :], v1[:, tt, :],
                                 sm[:, SM_BKV:SM_BKV + D])
        ln1T = persist.tile([P, DS, T], BF16, name="ln1T", tag="lnT")
        layernorm_to_T(v1, sm[:, SM_LN1G:SM_LN1G + DS],
                       sm[:, SM_LN1B:SM_LN1B + DS], ln1T, "ln1")
        warmers(10)

        x1T = persist.tile([P, CS, T], BF16, name="x1T", tag="xT")
        for cs_ in range(CS):
            pp = psacc.tile([P, T], F32, name="h1pp", tag="acc")
            for es in range(DS):
                nc.tensor.matmul(pp, wh1[:, es, cs_ * P:(cs_ + 1) * P],
                                 ln1T[:, es, :],
                                 start=(es == 0), stop=(es == DS - 1))
            nc.vector.tensor_scalar(
                x1T[:, cs_, :], pp, sm[:, SM_BH1 + cs_:SM_BH1 + cs_ + 1], 0.0,
                op0=ALU.add, op1=ALU.max)

        wo1 = wpool.tile([P, CS, D], BF16, name="wo1", tag="w")
        nc.sync.dma_start(wo1, wo1p[:, :, :])
        o1accs = [psacc.tile([P, D], F32, name=f"o1a{t}", tag="acc")
                  for t in range(TS)]
        for cs_ in range(CS):      # cs_-outer: o1 starts on x1T chunk 0,
            for tt in range(TS):   # overlapping the rest of the h1 phase
                nc.tensor.matmul(o1accs[tt],
                                 x1T[:, cs_, tt * P:(tt + 1) * P],
                                 wo1[:, cs_, :],
                                 start=(cs_ == 0), stop=(cs_ == CS - 1))
        for tt in range(TS):
            fin = work.tile([P, D], F32, name="fin", tag="fin")
            nc.vector.tensor_add(fin, o1accs[tt], v1[:, tt, :])
            nc.vector.tensor_add(fin, fin, sm[:, SM_BO1:SM_BO1 + D])
            nc.sync.dma_start(out[:, tt, :], fin)

    if not nc.is_finalized():
        nc.finalize()
    return nc


def _pcol(v):
    """[D] -> [128, D//128] partition-major columns."""
    return np.ascontiguousarray(v.reshape(-1, P).T)


def _pmajor(w, rows_per_part):
    """[(s p), c] -> [128, s, c]."""
    s = rows_per_part
    return np.ascontiguousarray(
        w.reshape(s, P, w.shape[1]).transpose(1, 0, 2))


def build_in_maps(inputs):
    f32 = lambda x: np.asarray(x, dtype=np.float32)
    bf = lambda x: np.ascontiguousarray(x).astype(NPBF)

    rel101 = f32(inputs["rel_enc"])[:L + 1]                     # [101, D]
    wkr = f32(inputs["wkr"])
    wb1 = f32(inputs["wb1"])
    RW = (rel101 @ wkr).T                                       # [D, 101]
    rwd8 = 8.0 * (RW[:, 1:] - RW[:, 0:1])                       # [D, 100]
    rwdT = rwd8.reshape(DS, P, BW).transpose(1, 0, 2).reshape(P, DS * BW)
    E1 = rel101 @ wkr @ wb1                                     # [101, H]
    e1d = (E1[1:] - E1[0:1]).T                                  # [H, 100]

    smalls = np.zeros((P, NS), np.float32)
    smalls[:, SM_LN0G:SM_LN0G + DS] = _pcol(f32(inputs["ln0_g"]))
    smalls[:, SM_LN0B:SM_LN0B + DS] = _pcol(f32(inputs["ln0_b"]))
    smalls[:, SM_LN1G:SM_LN1G + DS] = _pcol(f32(inputs["ln1_g"]))
    smalls[:, SM_LN1B:SM_LN1B + DS] = _pcol(f32(inputs["ln1_b"]))
    smalls[:, SM_BH0:SM_BH0 + CS] = _pcol(f32(inputs["b_h0"]))
    smalls[:, SM_BH1:SM_BH1 + CS] = _pcol(f32(inputs["b_h1"]))
    smalls[:, SM_BQ:SM_BQ + DS] = _pcol(f32(inputs["bq"]))  # (acc+bq)*0.125
    smalls[:, SM_BKE:SM_BKE + DS] = _pcol(f32(inputs["bke"]))
    smalls[:, SM_ID32:SM_ID32 + P] = np.eye(P, dtype=np.float32)
    smalls[:, SM_BKV:SM_BKV + D] = np.tile(f32(inputs["bkv"]), (P, 1))
    smalls[:, SM_BO1:SM_BO1 + D] = np.tile(f32(inputs["b_o1"]), (P, 1))

    mask = np.asarray(inputs["values_mask"])
    maskbias = np.where(mask, 0.0, NEG).astype(np.float32)      # [B, T]

    smb_base = np.zeros((P, NSB), np.float32)
    smb_base[:, SB_ID16:SB_ID16 + P] = np.eye(P, dtype=np.float32)
    smb_base[0, SB_ONES:SB_ONES + P] = 1.0
    smb_base[:, SB_RWD:SB_RWD + DS * BW] = rwdT
    smb_base[0, SB_E1D:SB_E1D + H * BW] = e1d.reshape(-1)
    smb_base[:, SB_WB0:SB_WB0 + DS * H] = _pmajor(f32(inputs["wb0"]), DS
                                                  ).reshape(P, DS * H)
    fill = np.ones(WROW, np.float32)   # multiplicative window: exp(0)=1
    fill[WIN:] = 0.0                   # causal mask: exact zero factor
    smb_base[:, SB_FILL:SB_FILL + WROW] = fill[None, :]

    shared = {
        "smalls": smalls,
        "wh0p": bf(_pmajor(f32(inputs["w_h0"]), DS)),
        "wqp": bf(_pmajor(f32(inputs["wq"]), CS)),
        "wkep": bf(_pmajor(f32(inputs["wke"]), CS)),
        "wkvp": bf(_pmajor(f32(inputs["wkv"]), CS)),
        "wh1p": bf(_pmajor(f32(inputs["w_h1"]), DS)),
        "wo1p": bf(_pmajor(f32(inputs["w_o1"]), CS)),
    }

    vals = f32(inputs["values"])
    in_maps = []
    for b in range(B):
        m = dict(shared)
        m["values_b"] = np.ascontiguousarray(
            vals[b].reshape(TS, P, D).transpose(1, 0, 2))
        smb = smb_base.copy()
        smb[0, SB_MASK:SB_MASK + T] = maskbias[b]
        m["smallsb_b"] = bf(smb)
        in_maps.append(m)
    return in_maps


_NC_CACHE = None


def kernel(**inputs) -> np.ndarray:
    global _NC_CACHE
    if _NC_CACHE is None:
        _NC_CACHE = build_nc()
    nc = _NC_CACHE

    from concourse.bass_utils import run_bass_kernel_spmd

    in_maps = build_in_maps(inputs)
    res = run_bass_kernel_spmd(nc, in_maps, core_ids=list(range(B)))
    return np.stack(
        [res.results[b]["out_b"].transpose(1, 0, 2).reshape(T, D)
         for b in range(B)], axis=0)


if __name__ == "__main__":
    nc = build_nc()
    print("built ok")


# revision 40
# speedup vs baseline: 1.2653x; 1.0285x over previous
"""Trainium2 Bass kernel for EncoderWithPositionalAttentionLayer.

Sharding: data-parallel over batch B=8 across 8 NeuronCores (one batch
element per core).  The batch-independent relative-position algebra is
collapsed on the HOST (exact fp32 numpy):

  score[i,j] = q[i].ke[j]/8 + q[i].RW[:,idx] + E1[idx,h] + b0[j] (+consts)
  idx = clip(j-i,-100,100)+100; under the causal mask idx in [0,100].
  Terms constant along a score row (idx=0 tables, bb0/bb1, bkr terms)
  cancel in softmax.  What remains is a banded bias
     Db[i,t] = (q[i]/8).(8*RWD[:,t]) + E1D[t,h],  t = j-i+100 in [1,100]
  with RWD/E1D host-computed delta tables (vs idx=0).

On device, exp(Db) goes through a DRAM scratch with read-side skew:
rows of width 360 per (partition, head, itile) hold
[127 ones][100 exp(Db)][133 zeros]; one contiguous write, then a read
with partition-dependent offset (stride ROWS-1) yields a j-aligned
window that multiplies each probability tile after the score exp
(exp(s+w) = exp(s)*exp(w)), applying band bias AND causal mask at once.
bias0 (+mask bias) enters the same way as a per-column factor
g[j] = exp(b0[j]) folded into kv, whose appended 65th column (= g)
makes attn @ kv_aug produce the softmax denominator for free.

Everything on the main path is bf16 (matmul rate is 1 cycle/row, same
as fp32r, at any moving dim; PSUM accumulation stays fp32).  Weights
are host-prepacked partition-major so every weight DMA is 128
contiguous 16KB runs.
"""

import contextlib
import sys

sys.path.insert(0, "/opt/trn_rl_repo")

import numpy as np
import ml_dtypes

import concourse.bass as bass
from concourse import bacc
import concourse.mybir as mybir
import concourse.tile as tile

F32 = mybir.dt.float32
BF16 = mybir.dt.bfloat16
AF = mybir.ActivationFunctionType
ALU = mybir.AluOpType
NPBF = ml_dtypes.bfloat16

B, T, D, H, HID = 8, 512, 512, 8, 2048
DIM = D // H          # 64
L = 100
BW = L                # band width (t = 1..100)
EPS = 1e-3
P = 128
TS = T // P           # 4
DS = D // P           # 4
CS = HID // P         # 16
NEG = -60.0           # exp(-60) ~ 1e-26: exact-enough masking
WROW = 360            # scratch row: [127 zeros][100 Db][133 NEG]
NK = H * TS           # 32 scratch tiles (k = ti*H + h)
ROWS = NK * WROW      # per-partition scratch row block (11520)
WIN = 227             # j-aligned window width read back per tile

# smalls (fp32) column offsets
SM_LN0G, SM_LN0B, SM_LN1G, SM_LN1B = 0, 4, 8, 12
SM_BH0, SM_BH1 = 16, 32
SM_BQ, SM_BKE = 48, 52
SM_ID32 = 56
SM_BKV = SM_ID32 + 128          # 184
SM_BO1 = SM_BKV + 512           # 696
NS = SM_BO1 + 512               # 1208

# smallsb (bf16) column offsets
SB_ID16 = 0
SB_ONES = 128
SB_RWD = 256                    # [128, 4*100]
SB_E1D = SB_RWD + 400           # row 0: 8 heads x 100
SB_MASK = SB_E1D + 800          # row 0: maskbias [T]
SB_WB0 = SB_MASK + 512          # [128, 4*8]
SB_FILL = SB_WB0 + 32           # [128, 360] scratch row fill pattern
NSB = SB_FILL + WROW            # 2388


def build_nc():
    nc = bacc.Bacc()

    dp = nc.declare_dram_parameter
    values = dp("values_b", [P, TS, D], F32, isOutput=False)
    smalls = dp("smalls", [P, NS], F32, isOutput=False)
    smallsb = dp("smallsb_b", [P, NSB], BF16, isOutput=False)
    wh0p = dp("wh0p", [P, DS, HID], BF16, isOutput=False)
    wqp = dp("wqp", [P, CS, D], BF16, isOutput=False)
    wkep = dp("wkep", [P, CS, D], BF16, isOutput=False)
    wkvp = dp("wkvp", [P, CS, D], BF16, isOutput=False)
    wh1p = dp("wh1p", [P, DS, HID], BF16, isOutput=False)
    wo1p = dp("wo1p", [P, CS, D], BF16, isOutput=False)
    out = dp("out_b", [P, TS, D], F32, isOutput=True)

    with tile.TileContext(nc) as tc, contextlib.ExitStack() as ctx:
        persist = ctx.enter_context(tc.tile_pool(name="persist", bufs=1))
        wpool = ctx.enter_context(tc.tile_pool(name="wpool", bufs=3))
        work = ctx.enter_context(tc.tile_pool(name="work", bufs=3))
        psum = ctx.enter_context(tc.tile_pool(name="psum", bufs=3, space="PSUM"))
        psacc = ctx.enter_context(tc.tile_pool(name="psacc", bufs=4, space="PSUM"))
        pwarm = ctx.enter_context(tc.tile_pool(name="pwarm", bufs=1, space="PSUM"))
        dram = ctx.enter_context(tc.tile_pool(name="dram", bufs=1, space="DRAM"))

        # ---------------- input DMAs ----------------------------------------
        # Everything early rides the sync ring: the scalar (ACT) engine
        # spends its first ~10us loading activation tables, which would
        # delay DMAs issued from it.  The scalar ring only carries the
        # mid-kernel scratch round-trip.
        # tiny boot DMA (identity + ones, 64KB) so PE warmers start ~1.5us in
        smbb = persist.tile([P, 256], BF16)
        nc.sync.dma_start(smbb, smallsb[:, 0:256])
        vals = persist.tile([P, TS, D], F32)
        nc.sync.dma_start(vals, values[:, :, :])
        sm = persist.tile([P, NS], F32)
        nc.sync.dma_start(sm, smalls[:, :])
        smb = persist.tile([P, NSB], BF16)
        nc.sync.dma_start(smb, smallsb[:, :])

        # ---------------- weight DMAs (sync ring, use order) -----------------
        wh0 = wpool.tile([P, DS, HID], BF16, name="wh0", tag="w")
        nc.sync.dma_start(wh0, wh0p[:, :, :])
        wq = wpool.tile([P, CS, D], BF16, name="wq", tag="w")
        nc.sync.dma_start(wq, wqp[:, :, :])
        wke = wpool.tile([P, CS, D], BF16, name="wke", tag="w")
        nc.sync.dma_start(wke, wkep[:, :, :])

        ident32 = sm[:, SM_ID32:SM_ID32 + 128]
        ident16 = smbb[:, 0:128]
        onesb = smbb[0:1, 128:256]
        rwdT = smb[:, SB_RWD:SB_RWD + 400].rearrange("p (s t) -> p s t", s=DS)
        wb0_sb = smb[:, SB_WB0:SB_WB0 + 32].rearrange("p (s h) -> p s h", s=DS)
        fill = smb[:, SB_FILL:SB_FILL + WROW]
        eps_sb = persist.tile([P, 1], F32)
        nc.vector.memset(eps_sb, EPS)

        # PE warmers: dependency-free matmuls that keep the HAM clock at
        # 8/8 while real matmul operands are still in flight.  One call is
        # a single accumulation group so the matmuls stream back-to-back
        # (per-matmul start/stop would serialize on the bank drain).
        warm_rhs = smbb[:, 0:256]

        def warmers(n):
            wps = pwarm.tile([P, 256], F32, name="warm", tag="warm")
            for i in range(n):
                nc.tensor.matmul(wps, ident16, warm_rhs,
                                 start=(i == 0), stop=(i == n - 1))

        warmers(50)

        # scratch fill+band SBUF image: [P, NK, WROW] bf16 (23KB/partition).
        # The fill copies are emitted later (just before the Db section) so
        # they queue behind the LN0 work on DVE instead of ahead of it.
        fb = persist.tile([P, NK, WROW], BF16)

        # --------------------------- LN helper --------------------------------
        def layernorm_to_T(x_tiles, gcol, bcol, lnT_out, name):
            for tt in range(TS):
                xt = x_tiles[:, tt, :]
                stats = work.tile([P, 6], F32, name=f"{name}st{tt}", tag="lnst")
                nc.vector.bn_stats(out=stats, in_=xt)
                mv = work.tile([P, 2], F32, name=f"{name}mv{tt}", tag="lnmv")
                nc.vector.bn_aggr(out=mv, in_=stats)
                rstd = work.tile([P, 1], F32, name=f"{name}rs{tt}", tag="lnrs")
                nc.scalar.activation(out=rstd, in_=mv[:, 1:2], func=AF.Sqrt,
                                     bias=eps_sb, scale=1.0)
                nc.vector.reciprocal(rstd, rstd)
                xn = work.tile([P, D], F32, name=f"{name}xn{tt}", tag="lnxn")
                nc.vector.tensor_scalar(xn, xt, mv[:, 0:1], rstd,
                                        op0=ALU.subtract, op1=ALU.mult)
                for es in range(DS):
                    tp = psum.tile([P, P], F32, name=f"{name}tp", tag="pp")
                    nc.tensor.transpose(tp, xn[:, es * P:(es + 1) * P], ident32)
                    nc.vector.tensor_scalar(
                        lnT_out[:, es, tt * P:(tt + 1) * P], tp,
                        gcol[:, es:es + 1], bcol[:, es:es + 1],
                        op0=ALU.mult, op1=ALU.add)

        # ------------------------- LN0 + block0 ------------------------------
        ln0T = persist.tile([P, DS, T], BF16, name="ln0T", tag="lnT")
        layernorm_to_T(vals, sm[:, SM_LN0G:SM_LN0G + DS],
                       sm[:, SM_LN0B:SM_LN0B + DS], ln0T, "ln0")

        xT = persist.tile([P, CS, T], BF16, name="xT", tag="xT")
        for cs_ in range(CS):
            pp = psacc.tile([P, T], F32, name="h0pp", tag="acc")
            for es in range(DS):
                nc.tensor.matmul(pp, wh0[:, es, cs_ * P:(cs_ + 1) * P],
                                 ln0T[:, es, :],
                                 start=(es == 0), stop=(es == DS - 1))
            nc.vector.tensor_scalar(      # fused bias + relu on DVE
                xT[:, cs_, :], pp, sm[:, SM_BH0 + cs_:SM_BH0 + cs_ + 1], 0.0,
                op0=ALU.add, op1=ALU.max)

        # --------------------------- projections -----------------------------
        def project_T(w_sb, dest, boff, scale):
            """dest [128(d), DS, T] (bf16) = scale*((x @ w).T + b)."""
            accs = [psacc.tile([P, T], F32, name=f"pa{d}", tag="acc")
                    for d in range(DS)]
            for cs_ in range(CS):
                for dsub in range(DS):
                    nc.tensor.matmul(
                        accs[dsub], w_sb[:, cs_, dsub * P:(dsub + 1) * P],
                        xT[:, cs_, :],
                        start=(cs_ == 0), stop=(cs_ == CS - 1))
            for dsub in range(DS):
                nc.vector.tensor_scalar(   # (acc + b) * scale on DVE
                    dest[:, dsub, :], accs[dsub],
                    sm[:, boff + dsub:boff + dsub + 1], scale,
                    op0=ALU.add, op1=ALU.mult)

        qT = persist.tile([P, DS, T], BF16)      # holds q/8 transposed
        project_T(wq, qT, SM_BQ, 0.125)          # bias pre-scaled on host

        # ---------------- Db tiles (interleaved with keT) ---------------------
        # fill margins: row layout [127 zeros][100 Db][133 NEG], two
        # broadcast copies over all NK rows
        nc.vector.tensor_copy(
            fb[:, :, 0:127],
            fill[:, 0:127].rearrange("p (a w) -> p a w", a=1)
                .to_broadcast((P, NK, 127)))
        nc.vector.tensor_copy(
            fb[:, :, WIN:WROW],
            fill[:, WIN:WROW].rearrange("p (a w) -> p a w", a=1)
                .to_broadcast((P, NK, WROW - WIN)))
        hd = lambda h: (h % 2) * DIM

        def db_group(ti, hh):
            """Db for heads 4*hh..4*hh+3 of row-tile ti into one PSUM bank."""
            dbp = psum.tile([P, 4, BW], F32, name="dbp", tag="pp")
            for i4 in range(4):
                h = hh * 4 + i4
                nc.tensor.matmul(
                    dbp[:, i4, :],
                    qT[hd(h):hd(h) + DIM, h // 2, ti * P:(ti + 1) * P],
                    rwdT[hd(h):hd(h) + DIM, h // 2, :],
                    start=True, stop=False)
                nc.tensor.matmul(
                    dbp[:, i4, :], onesb,
                    smb[0:1, SB_E1D + h * BW:SB_E1D + (h + 1) * BW],
                    start=False, stop=True)
            # store exp(Db): the window is applied multiplicatively after
            # the score exp (exp(s+w) = exp(s)*exp(w)); fill is 1.0 / 0.0
            nc.scalar.activation(
                out=fb[:, ti * H + hh * 4:ti * H + hh * 4 + 4, 127:227],
                in_=dbp, func=AF.Exp, bias=0.0, scale=1.0)

        # keT projection with Db groups interleaved so the PE array duty
        # cycle stays high (Db matmuls alone are LDW-dominated)
        keT = persist.tile([P, DS, T], BF16)
        keaccs = [psacc.tile([P, T], F32, name=f"kea{d}", tag="acc")
                  for d in range(DS)]
        for cs_ in range(CS):
            for dsub in range(DS):
                nc.tensor.matmul(
                    keaccs[dsub], wke[:, cs_, dsub * P:(dsub + 1) * P],
                    xT[:, cs_, :],
                    start=(cs_ == 0), stop=(cs_ == CS - 1))
            if cs_ % 2 == 1:
                db_group((cs_ - 1) // 4, ((cs_ - 1) // 2) % 2)
        for dsub in range(DS):
            nc.vector.tensor_scalar(
                keT[:, dsub, :], keaccs[dsub],
                sm[:, SM_BKE + dsub:SM_BKE + dsub + 1], 1.0,
                op0=ALU.add, op1=ALU.mult)

        scr = dram.tile([P * ROWS], BF16, name="scr")
        nc.scalar.dma_start(
            bass.AP(tensor=scr.tensor, offset=scr.offset,
                    ap=[[ROWS, P], [WROW, NK], [1, WROW]]),
            fb)
        # skewed window read: win[p, k, w] = scr[p*ROWS + k*WROW + 127 + w - p]
        wins = []
        for ti in range(TS):
            wr = work.tile([P, H, WIN], BF16, name=f"win{ti}", tag=f"win{ti}",
                           bufs=1)
            nc.scalar.dma_start(
                wr,
                bass.AP(tensor=scr.tensor,
                        offset=scr.offset + ti * H * WROW + 127,
                        ap=[[ROWS - 1, P], [WROW, H], [1, WIN]]))
            wins.append(wr)

        # bias0 (+maskbias) enters the softmax as a per-column factor
        # g[j] = exp(b0[j] + maskbias[j]): fold it into kv, with a 65th
        # column equal to g so attn@kv_aug yields the softmax denominator.
        b0p = psum.tile([H, T], F32, name="b0p", tag="pp")
        for c in range(DS):
            nc.tensor.matmul(b0p, wb0_sb[:, c, :], keT[:, c, :],
                             start=(c == 0), stop=False)
        nc.tensor.matmul(b0p, onesb[:, 0:H], smb[0:1, SB_MASK:SB_MASK + T],
                         start=False, stop=True)
        b0m = work.tile([H, T], F32, name="b0m", tag="b0m", bufs=1)
        nc.vector.tensor_copy(b0m, b0p)
        g = persist.tile([P, TS, H], F32)
        for tt in range(TS):
            gp = psum.tile([P, H], F32, name="gp", tag="pp")
            nc.tensor.matmul(gp, b0m[:, tt * P:(tt + 1) * P],
                             ident32[0:H, 0:H], start=True, stop=True)
            nc.scalar.activation(out=g[:, tt, :], in_=gp, func=AF.Exp,
                                 bias=0.0, scale=1.0)

        wkv = wpool.tile([P, CS, D], BF16, name="wkv", tag="w")
        nc.sync.dma_start(wkv, wkvp[:, :, :])
        kva = persist.tile([P, TS, H, DIM + 1], BF16)
        kvaccs = [psacc.tile([P, D], F32, name=f"kva{t}", tag="acc")
                  for t in range(TS)]
        for cs_ in range(CS):
            for tt in range(TS):
                nc.tensor.matmul(kvaccs[tt],
                                 xT[:, cs_, tt * P:(tt + 1) * P],
                                 wkv[:, cs_, :],
                                 start=(cs_ == 0), stop=(cs_ == CS - 1))
        for tt in range(TS):
            for h in range(H):       # bkv itself is applied via v1 below
                nc.scalar.activation(
                    out=kva[:, tt, h, 0:DIM],
                    in_=kvaccs[tt][:, h * DIM:(h + 1) * DIM],
                    func=AF.Identity, bias=0.0, scale=g[:, tt, h:h + 1])
            nc.vector.tensor_copy(kva[:, tt, :, DIM], g[:, tt, :])

        # ------------------------------ attention -----------------------------
        # Per ti, three phases so the PE queue never blocks on DVE/ACT:
        # (1) all 8 heads' score matmuls (+DVE bias adds, ACT exp),
        # (2) all probability-tile transposes (DVE copies trail),
        # (3) all attn@v accumulation matmuls (+ACT rz scale-out).
        attn_out = persist.tile([P, TS, D], F32)
        for ti in range(TS):
            nj = (ti + 1) * P
            j0 = ti * P - 99
            warmers(8 if ti < 2 else 12)
            ats = []
            for h in range(H):
                sp = psacc.tile([P, T], F32, name="sp", tag="acc")
                nc.tensor.matmul(
                    sp[:, 0:nj],
                    qT[hd(h):hd(h) + DIM, h // 2, ti * P:(ti + 1) * P],
                    keT[hd(h):hd(h) + DIM, h // 2, 0:nj],
                    start=True, stop=True)
                at = work.tile([P, T], BF16, name=f"at{h}", tag=f"at{h}",
                               bufs=1)
                nc.scalar.activation(out=at[:, 0:nj], in_=sp[:, 0:nj],
                                     func=AF.Exp, bias=0.0, scale=1.0)
                # banded bias and causal mask enter multiplicatively:
                # window holds exp(Db) in the band, 1.0 below, 0.0 above diag
                if ti == 0:
                    nc.vector.tensor_tensor(at[:, 0:P], at[:, 0:P],
                                            wins[0][:, h, 99:WIN], ALU.mult)
                else:
                    nc.vector.tensor_tensor(at[:, j0:j0 + WIN],
                                            at[:, j0:j0 + WIN],
                                            wins[ti][:, h, :], ALU.mult)
                ats.append(at)
            if ti >= 2:
                warmers(6)
            atTs = work.tile([P, H, (ti + 1) * P], BF16, name="atTs",
                             tag="atTs", bufs=2)
            for h in range(H):
                tp = psum.tile([P, ti + 1, P], BF16, name="attp", tag="pp")
                for js in range(ti + 1):
                    nc.tensor.transpose(tp[:, js, :],
                                        ats[h][:, js * P:(js + 1) * P],
                                        ident16)
                nc.vector.tensor_copy(
                    atTs[:, h, :].rearrange("p (a b) -> p a b", a=ti + 1), tp)
            if ti >= 2:
                warmers(6)
            for h in range(H):
                op = psacc.tile([P, DIM + 1], F32, name="avp", tag="acc")
                for js in range(ti + 1):
                    nc.tensor.matmul(op, atTs[:, h, js * P:(js + 1) * P],
                                     kva[:, js, h, :],
                                     start=(js == 0), stop=(js == ti))
                rz = work.tile([P, 1], F32, name="rz", tag="rz")
                nc.vector.reciprocal(rz, op[:, DIM:DIM + 1])
                nc.scalar.activation(
                    out=attn_out[:, ti, h * DIM:(h + 1) * DIM],
                    in_=op[:, 0:DIM],
                    func=AF.Identity, bias=0.0, scale=rz)

        # ------------------------ residual + block1 ---------------------------
        wh1 = wpool.tile([P, DS, HID], BF16, name="wh1", tag="w")
        nc.sync.dma_start(wh1, wh1p[:, :, :])
        v1 = persist.tile([P, TS, D], F32)
        warmers(12)
        for tt in range(TS):
            nc.vector.tensor_add(v1[:, tt, :], vals[:, tt, :],
                                 attn_out[:, tt, :])
            nc.vector.tensor_add(v1[:, tt, :], v1[:, tt, :],
                                 sm[:, SM_BKV:SM_BKV + D])
        ln1T = persist.tile([P, DS, T], BF16, name="ln1T", tag="lnT")
        layernorm_to_T(v1, sm[:, SM_LN1G:SM_LN1G + DS],
                       sm[:, SM_LN1B:SM_LN1B + DS], ln1T, "ln1")
        warmers(10)

        x1T = persist.tile([P, CS, T], BF16, name="x1T", tag="xT")
        for cs_ in range(CS):
            pp = psacc.tile([P, T], F32, name="h1pp", tag="acc")
            for es in range(DS):
                nc.tensor.matmul(pp, wh1[:, es, cs_ * P:(cs_ + 1) * P],
                                 ln1T[:, es, :],
                                 start=(es == 0), stop=(es == DS - 1))
            nc.vector.tensor_scalar(
                x1T[:, cs_, :], pp, sm[:, SM_BH1 + cs_:SM_BH1 + cs_ + 1], 0.0,
                op0=ALU.add, op1=ALU.max)

        wo1 = wpool.tile([P, CS, D], BF16, name="wo1", tag="w")
        nc.sync.dma_start(wo1, wo1p[:, :, :])
        o1accs = [psacc.tile([P, D], F32, name=f"o1a{t}", tag="acc")
                  for t in range(TS)]
        for cs_ in range(CS):      # cs_-outer: o1 starts on x1T chunk 0,
            for tt in range(TS):   # overlapping the rest of the h1 phase
                nc.tensor.matmul(o1accs[tt],
                                 x1T[:, cs_, tt * P:(tt + 1) * P],
                                 wo1[:, cs_, :],
                                 start=(cs_ == 0), stop=(cs_ == CS - 1))
        for tt in range(TS):
            fin = work.tile([P, D], F32, name="fin", tag="fin")
            nc.vector.tensor_add(fin, o1accs[tt], v1[:, tt, :])
            nc.vector.tensor_add(fin, fin, sm[:, SM_BO1:SM_BO1 + D])
            nc.sync.dma_start(out[:, tt, :], fin)

    if not nc.is_finalized():
        nc.finalize()
    return nc


def _pcol(v):
    """[D] -> [128, D//128] partition-major columns."""
    return np.ascontiguousarray(v.reshape(-1, P).T)


def _pmajor(w, rows_per_part):
    """[(s p), c] -> [128, s, c]."""
    s = rows_per_part
    return np.ascontiguousarray(
        w.reshape(s, P, w.shape[1]).transpose(1, 0, 2))


def build_in_maps(inputs):
    f32 = lambda x: np.asarray(x, dtype=np.float32)
    bf = lambda x: np.ascontiguousarray(x).astype(NPBF)

    rel101 = f32(inputs["rel_enc"])[:L + 1]                     # [101, D]
    wkr = f32(inputs["wkr"])
    wb1 = f32(inputs["wb1"])
    RW = (rel101 @ wkr).T                                       # [D, 101]
    rwd8 = 8.0 * (RW[:, 1:] - RW[:, 0:1])                       # [D, 100]
    rwdT = rwd8.reshape(DS, P, BW).transpose(1, 0, 2).reshape(P, DS * BW)
    E1 = rel101 @ wkr @ wb1                                     # [101, H]
    e1d = (E1[1:] - E1[0:1]).T                                  # [H, 100]

    smalls = np.zeros((P, NS), np.float32)
    smalls[:, SM_LN0G:SM_LN0G + DS] = _pcol(f32(inputs["ln0_g"]))
    smalls[:, SM_LN0B:SM_LN0B + DS] = _pcol(f32(inputs["ln0_b"]))
    smalls[:, SM_LN1G:SM_LN1G + DS] = _pcol(f32(inputs["ln1_g"]))
    smalls[:, SM_LN1B:SM_LN1B + DS] = _pcol(f32(inputs["ln1_b"]))
    smalls[:, SM_BH0:SM_BH0 + CS] = _pcol(f32(inputs["b_h0"]))
    smalls[:, SM_BH1:SM_BH1 + CS] = _pcol(f32(inputs["b_h1"]))
    smalls[:, SM_BQ:SM_BQ + DS] = _pcol(f32(inputs["bq"]))  # (acc+bq)*0.125
    smalls[:, SM_BKE:SM_BKE + DS] = _pcol(f32(inputs["bke"]))
    smalls[:, SM_ID32:SM_ID32 + P] = np.eye(P, dtype=np.float32)
    smalls[:, SM_BKV:SM_BKV + D] = np.tile(f32(inputs["bkv"]), (P, 1))
    smalls[:, SM_BO1:SM_BO1 + D] = np.tile(f32(inputs["b_o1"]), (P, 1))

    mask = np.asarray(inputs["values_mask"])
    maskbias = np.where(mask, 0.0, NEG).astype(np.float32)      # [B, T]

    smb_base = np.zeros((P, NSB), np.float32)
    smb_base[:, SB_ID16:SB_ID16 + P] = np.eye(P, dtype=np.float32)
    smb_base[0, SB_ONES:SB_ONES + P] = 1.0
    smb_base[:, SB_RWD:SB_RWD + DS * BW] = rwdT
    smb_base[0, SB_E1D:SB_E1D + H * BW] = e1d.reshape(-1)
    smb_base[:, SB_WB0:SB_WB0 + DS * H] = _pmajor(f32(inputs["wb0"]), DS
                                                  ).reshape(P, DS * H)
    fill = np.ones(WROW, np.float32)   # multiplicative window: exp(0)=1
    fill[WIN:] = 0.0                   # causal mask: exact zero factor
    smb_base[:, SB_FILL:SB_FILL + WROW] = fill[None, :]

    shared = {
        "smalls": smalls,
        "wh0p": bf(_pmajor(f32(inputs["w_h0"]), DS)),
        "wqp": bf(_pmajor(f32(inputs["wq"]), CS)),
        "wkep": bf(_pmajor(f32(inputs["wke"]), CS)),
        "wkvp": bf(_pmajor(f32(inputs["wkv"]), CS)),
        "wh1p": bf(_pmajor(f32(inputs["w_h1"]), DS)),
        "wo1p": bf(_pmajor(f32(inputs["w_o1"]), CS)),
    }

    vals = f32(inputs["values"])
    in_maps = []
    for b in range(B):
        m = dict(shared)
        m["values_b"] = np.ascontiguousarray(
            vals[b].reshape(TS, P, D).transpose(1, 0, 2))
        smb = smb_base.copy()
        smb[0, SB_MASK:SB_MASK + T] = maskbias[b]
        m["smallsb_b"] = bf(smb)
        in_maps.append(m)
    return in_maps


_NC_CACHE = None


def kernel(**inputs) -> np.ndarray:
    global _NC_CACHE
    if _NC_CACHE is None:
        _NC_CACHE = build_nc()
    nc = _NC_CACHE

    from concourse.bass_utils import run_bass_kernel_spmd

    in_maps = build_in_maps(inputs)
    res = run_bass_kernel_spmd(nc, in_maps, core_ids=list(range(B)))
    return np.stack(
        [res.results[b]["out_b"].transpose(1, 0, 2).reshape(T, D)
         for b in range(B)], axis=0)


if __name__ == "__main__":
    nc = build_nc()
    print("built ok")


# revision 41
# speedup vs baseline: 1.2874x; 1.0175x over previous
"""Trainium2 Bass kernel for EncoderWithPositionalAttentionLayer.

Sharding: data-parallel over batch B=8 across 8 NeuronCores (one batch
element per core).  The batch-independent relative-position algebra is
collapsed on the HOST (exact fp32 numpy):

  score[i,j] = q[i].ke[j]/8 + q[i].RW[:,idx] + E1[idx,h] + b0[j] (+consts)
  idx = clip(j-i,-100,100)+100; under the causal mask idx in [0,100].
  Terms constant along a score row (idx=0 tables, bb0/bb1, bkr terms)
  cancel in softmax.  What remains is a banded bias
     Db[i,t] = (q[i]/8).(8*RWD[:,t]) + E1D[t,h],  t = j-i+100 in [1,100]
  with RWD/E1D host-computed delta tables (vs idx=0).

On device, exp(Db) goes through a DRAM scratch with read-side skew:
rows of width 360 per (partition, head, itile) hold
[127 ones][100 exp(Db)][133 zeros]; one contiguous write, then a read
with partition-dependent offset (stride ROWS-1) yields a j-aligned
window that multiplies each probability tile after the score exp
(exp(s+w) = exp(s)*exp(w)), applying band bias AND causal mask at once.
bias0 (+mask bias) enters the same way as a per-column factor
g[j] = exp(b0[j]) folded into kv, whose appended 65th column (= g)
makes attn @ kv_aug produce the softmax denominator for free.

Everything on the main path is bf16 (matmul rate is 1 cycle/row, same
as fp32r, at any moving dim; PSUM accumulation stays fp32).  Weights
are host-prepacked partition-major so every weight DMA is 128
contiguous 16KB runs.
"""

import contextlib
import sys

sys.path.insert(0, "/opt/trn_rl_repo")

import numpy as np
import ml_dtypes

import concourse.bass as bass
from concourse import bacc
import concourse.mybir as mybir
import concourse.tile as tile

F32 = mybir.dt.float32
BF16 = mybir.dt.bfloat16
AF = mybir.ActivationFunctionType
ALU = mybir.AluOpType
NPBF = ml_dtypes.bfloat16

B, T, D, H, HID = 8, 512, 512, 8, 2048
DIM = D // H          # 64
L = 100
BW = L                # band width (t = 1..100)
EPS = 1e-3
P = 128
TS = T // P           # 4
DS = D // P           # 4
CS = HID // P         # 16
NEG = -60.0           # exp(-60) ~ 1e-26: exact-enough masking
WROW = 360            # scratch row: [127 zeros][100 Db][133 NEG]
NK = H * TS           # 32 scratch tiles (k = ti*H + h)
ROWS = NK * WROW      # per-partition scratch row block (11520)
WIN = 227             # j-aligned window width read back per tile

# smalls (fp32) column offsets
SM_LN0G, SM_LN0B, SM_LN1G, SM_LN1B = 0, 4, 8, 12
SM_BH0, SM_BH1 = 16, 32
SM_BQ, SM_BKE = 48, 52
SM_ID32 = 56
SM_BKV = SM_ID32 + 128          # 184
SM_BO1 = SM_BKV + 512           # 696
NS = SM_BO1 + 512               # 1208

# smallsb (bf16) column offsets
SB_ID16 = 0
SB_ONES = 128
SB_RWD = 256                    # [128, 4*100]
SB_E1D = SB_RWD + 400           # row 0: 8 heads x 100
SB_MASK = SB_E1D + 800          # row 0: maskbias [T]
SB_WB0 = SB_MASK + 512          # [128, 4*8]
SB_FILL = SB_WB0 + 32           # [128, 360] scratch row fill pattern
NSB = SB_FILL + WROW            # 2388


def build_nc():
    nc = bacc.Bacc()

    dp = nc.declare_dram_parameter
    values = dp("values_b", [P, TS, D], F32, isOutput=False)
    smalls = dp("smalls", [P, NS], F32, isOutput=False)
    smallsb = dp("smallsb_b", [P, NSB], BF16, isOutput=False)
    wh0p = dp("wh0p", [P, DS, HID], BF16, isOutput=False)
    wqp = dp("wqp", [P, CS, D], BF16, isOutput=False)
    wkep = dp("wkep", [P, CS, D], BF16, isOutput=False)
    wkvp = dp("wkvp", [P, CS, D], BF16, isOutput=False)
    wh1p = dp("wh1p", [P, DS, HID], BF16, isOutput=False)
    wo1p = dp("wo1p", [P, CS, D], BF16, isOutput=False)
    out = dp("out_b", [P, TS, D], F32, isOutput=True)

    with tile.TileContext(nc) as tc, contextlib.ExitStack() as ctx:
        persist = ctx.enter_context(tc.tile_pool(name="persist", bufs=1))
        wpool = ctx.enter_context(tc.tile_pool(name="wpool", bufs=3))
        work = ctx.enter_context(tc.tile_pool(name="work", bufs=3))
        psum = ctx.enter_context(tc.tile_pool(name="psum", bufs=3, space="PSUM"))
        psacc = ctx.enter_context(tc.tile_pool(name="psacc", bufs=4, space="PSUM"))
        pwarm = ctx.enter_context(tc.tile_pool(name="pwarm", bufs=1, space="PSUM"))
        dram = ctx.enter_context(tc.tile_pool(name="dram", bufs=1, space="DRAM"))

        # ---------------- input DMAs ----------------------------------------
        # Everything early rides the sync ring: the scalar (ACT) engine
        # spends its first ~10us loading activation tables, which would
        # delay DMAs issued from it.  The scalar ring only carries the
        # mid-kernel scratch round-trip.
        # tiny boot DMA (identity + ones, 64KB) so PE warmers start ~1.5us in
        smbb = persist.tile([P, 256], BF16)
        nc.sync.dma_start(smbb, smallsb[:, 0:256])
        vals = persist.tile([P, TS, D], F32)
        nc.sync.dma_start(vals, values[:, :, :])
        sm = persist.tile([P, NS], F32)
        nc.sync.dma_start(sm, smalls[:, :])
        smb = persist.tile([P, NSB], BF16)
        nc.sync.dma_start(smb, smallsb[:, :])

        # ---------------- weight DMAs (sync ring, use order) -----------------
        wh0 = wpool.tile([P, DS, HID], BF16, name="wh0", tag="w")
        nc.sync.dma_start(wh0, wh0p[:, :, :])
        wq = wpool.tile([P, CS, D], BF16, name="wq", tag="w")
        nc.sync.dma_start(wq, wqp[:, :, :])
        wke = wpool.tile([P, CS, D], BF16, name="wke", tag="w")
        nc.sync.dma_start(wke, wkep[:, :, :])

        ident32 = sm[:, SM_ID32:SM_ID32 + 128]
        ident16 = smbb[:, 0:128]
        onesb = smbb[0:1, 128:256]
        rwdT = smb[:, SB_RWD:SB_RWD + 400].rearrange("p (s t) -> p s t", s=DS)
        wb0_sb = smb[:, SB_WB0:SB_WB0 + 32].rearrange("p (s h) -> p s h", s=DS)
        fill = smb[:, SB_FILL:SB_FILL + WROW]
        eps_sb = persist.tile([P, 1], F32)
        nc.vector.memset(eps_sb, EPS)

        # PE warmers: dependency-free matmuls that keep the HAM clock at
        # 8/8 while real matmul operands are still in flight.  One call is
        # a single accumulation group so the matmuls stream back-to-back
        # (per-matmul start/stop would serialize on the bank drain).
        warm_rhs = smbb[:, 0:256]

        def warmers(n):
            wps = pwarm.tile([P, 256], F32, name="warm", tag="warm")
            for i in range(n):
                nc.tensor.matmul(wps, ident16, warm_rhs,
                                 start=(i == 0), stop=(i == n - 1))

        warmers(50)

        # scratch fill+band SBUF image: [P, NK, WROW] bf16 (23KB/partition).
        # The fill copies are emitted later (just before the Db section) so
        # they queue behind the LN0 work on DVE instead of ahead of it.
        fb = persist.tile([P, NK, WROW], BF16)

        # --------------------------- LN helper --------------------------------
        def layernorm_to_T(x_tiles, gcol, bcol, lnT_out, name):
            for tt in range(TS):
                xt = x_tiles[:, tt, :]
                stats = work.tile([P, 6], F32, name=f"{name}st{tt}", tag="lnst")
                nc.vector.bn_stats(out=stats, in_=xt)
                mv = work.tile([P, 2], F32, name=f"{name}mv{tt}", tag="lnmv")
                nc.vector.bn_aggr(out=mv, in_=stats)
                rstd = work.tile([P, 1], F32, name=f"{name}rs{tt}", tag="lnrs")
                nc.scalar.activation(out=rstd, in_=mv[:, 1:2], func=AF.Sqrt,
                                     bias=eps_sb, scale=1.0)
                nc.vector.reciprocal(rstd, rstd)
                xn = work.tile([P, D], F32, name=f"{name}xn{tt}", tag="lnxn")
                nc.vector.tensor_scalar(xn, xt, mv[:, 0:1], rstd,
                                        op0=ALU.subtract, op1=ALU.mult)
                for es in range(DS):
                    tp = psum.tile([P, P], F32, name=f"{name}tp", tag="pp")
                    nc.tensor.transpose(tp, xn[:, es * P:(es + 1) * P], ident32)
                    nc.vector.tensor_scalar(
                        lnT_out[:, es, tt * P:(tt + 1) * P], tp,
                        gcol[:, es:es + 1], bcol[:, es:es + 1],
                        op0=ALU.mult, op1=ALU.add)

        # ------------------------- LN0 + block0 ------------------------------
        ln0T = persist.tile([P, DS, T], BF16, name="ln0T", tag="lnT")
        layernorm_to_T(vals, sm[:, SM_LN0G:SM_LN0G + DS],
                       sm[:, SM_LN0B:SM_LN0B + DS], ln0T, "ln0")

        xT = persist.tile([P, CS, T], BF16, name="xT", tag="xT")
        for cs_ in range(CS):
            pp = psacc.tile([P, T], F32, name="h0pp", tag="acc")
            for es in range(DS):
                nc.tensor.matmul(pp, wh0[:, es, cs_ * P:(cs_ + 1) * P],
                                 ln0T[:, es, :],
                                 start=(es == 0), stop=(es == DS - 1))
            nc.vector.tensor_scalar(      # fused bias + relu on DVE
                xT[:, cs_, :], pp, sm[:, SM_BH0 + cs_:SM_BH0 + cs_ + 1], 0.0,
                op0=ALU.add, op1=ALU.max)

        # --------------------------- projections -----------------------------
        def project_T(w_sb, dest, boff, scale):
            """dest [128(d), DS, T] (bf16) = scale*((x @ w).T + b)."""
            accs = [psacc.tile([P, T], F32, name=f"pa{d}", tag="acc")
                    for d in range(DS)]
            for cs_ in range(CS):
                for dsub in range(DS):
                    nc.tensor.matmul(
                        accs[dsub], w_sb[:, cs_, dsub * P:(dsub + 1) * P],
                        xT[:, cs_, :],
                        start=(cs_ == 0), stop=(cs_ == CS - 1))
            for dsub in range(DS):
                nc.vector.tensor_scalar(   # (acc + b) * scale on DVE
                    dest[:, dsub, :], accs[dsub],
                    sm[:, boff + dsub:boff + dsub + 1], scale,
                    op0=ALU.add, op1=ALU.mult)

        qT = persist.tile([P, DS, T], BF16)      # holds q/8 transposed
        project_T(wq, qT, SM_BQ, 0.125)          # bias pre-scaled on host

        # ---------------- Db tiles (interleaved with keT) ---------------------
        # fill margins: row layout [127 zeros][100 Db][133 NEG], two
        # broadcast copies over all NK rows
        nc.vector.tensor_copy(
            fb[:, :, 0:127],
            fill[:, 0:127].rearrange("p (a w) -> p a w", a=1)
                .to_broadcast((P, NK, 127)))
        nc.vector.tensor_copy(
            fb[:, :, WIN:WROW],
            fill[:, WIN:WROW].rearrange("p (a w) -> p a w", a=1)
                .to_broadcast((P, NK, WROW - WIN)))
        hd = lambda h: (h % 2) * DIM

        def db_group(ti, hh):
            """Db for heads 4*hh..4*hh+3 of row-tile ti into one PSUM bank."""
            dbp = psum.tile([P, 4, BW], F32, name="dbp", tag="pp")
            for i4 in range(4):
                h = hh * 4 + i4
                nc.tensor.matmul(
                    dbp[:, i4, :],
                    qT[hd(h):hd(h) + DIM, h // 2, ti * P:(ti + 1) * P],
                    rwdT[hd(h):hd(h) + DIM, h // 2, :],
                    start=True, stop=False)
                nc.tensor.matmul(
                    dbp[:, i4, :], onesb,
                    smb[0:1, SB_E1D + h * BW:SB_E1D + (h + 1) * BW],
                    start=False, stop=True)
            # store exp(Db): the window is applied multiplicatively after
            # the score exp (exp(s+w) = exp(s)*exp(w)); fill is 1.0 / 0.0
            nc.scalar.activation(
                out=fb[:, ti * H + hh * 4:ti * H + hh * 4 + 4, 127:227],
                in_=dbp, func=AF.Exp, bias=0.0, scale=1.0)

        # keT projection with Db groups interleaved so the PE array duty
        # cycle stays high (Db matmuls alone are LDW-dominated)
        keT = persist.tile([P, DS, T], BF16)
        keaccs = [psacc.tile([P, T], F32, name=f"kea{d}", tag="acc")
                  for d in range(DS)]
        for cs_ in range(CS):
            for dsub in range(DS):
                nc.tensor.matmul(
                    keaccs[dsub], wke[:, cs_, dsub * P:(dsub + 1) * P],
                    xT[:, cs_, :],
                    start=(cs_ == 0), stop=(cs_ == CS - 1))
            if cs_ % 2 == 1:
                db_group((cs_ - 1) // 4, ((cs_ - 1) // 2) % 2)
        for dsub in range(DS):
            nc.vector.tensor_scalar(
                keT[:, dsub, :], keaccs[dsub],
                sm[:, SM_BKE + dsub:SM_BKE + dsub + 1], 1.0,
                op0=ALU.add, op1=ALU.mult)

        scr = dram.tile([P * ROWS], BF16, name="scr")
        nc.scalar.dma_start(
            bass.AP(tensor=scr.tensor, offset=scr.offset,
                    ap=[[ROWS, P], [WROW, NK], [1, WROW]]),
            fb)
        # skewed window read: win[p, k, w] = scr[p*ROWS + k*WROW + 127 + w - p]
        wins = []
        for ti in range(TS):
            wr = work.tile([P, H, WIN], BF16, name=f"win{ti}", tag=f"win{ti}",
                           bufs=1)
            nc.scalar.dma_start(
                wr,
                bass.AP(tensor=scr.tensor,
                        offset=scr.offset + ti * H * WROW + 127,
                        ap=[[ROWS - 1, P], [WROW, H], [1, WIN]]))
            wins.append(wr)

        # bias0 (+maskbias) enters the softmax as a per-column factor
        # g[j] = exp(b0[j] + maskbias[j]): fold it into kv, with a 65th
        # column equal to g so attn@kv_aug yields the softmax denominator.
        b0p = psum.tile([H, T], F32, name="b0p", tag="pp")
        for c in range(DS):
            nc.tensor.matmul(b0p, wb0_sb[:, c, :], keT[:, c, :],
                             start=(c == 0), stop=False)
        nc.tensor.matmul(b0p, onesb[:, 0:H], smb[0:1, SB_MASK:SB_MASK + T],
                         start=False, stop=True)
        b0m = work.tile([H, T], F32, name="b0m", tag="b0m", bufs=1)
        nc.vector.tensor_copy(b0m, b0p)
        g = persist.tile([P, TS, H], F32)
        for tt in range(TS):
            gp = psum.tile([P, H], F32, name="gp", tag="pp")
            nc.tensor.matmul(gp, b0m[:, tt * P:(tt + 1) * P],
                             ident32[0:H, 0:H], start=True, stop=True)
            nc.scalar.activation(out=g[:, tt, :], in_=gp, func=AF.Exp,
                                 bias=0.0, scale=1.0)

        wkv = wpool.tile([P, CS, D], BF16, name="wkv", tag="w")
        nc.sync.dma_start(wkv, wkvp[:, :, :])
        kva = persist.tile([P, TS, H, DIM + 1], BF16)
        kvaccs = [psacc.tile([P, D], F32, name=f"kva{t}", tag="acc")
                  for t in range(TS)]
        for cs_ in range(CS):
            for tt in range(TS):
                nc.tensor.matmul(kvaccs[tt],
                                 xT[:, cs_, tt * P:(tt + 1) * P],
                                 wkv[:, cs_, :],
                                 start=(cs_ == 0), stop=(cs_ == CS - 1))
        for tt in range(TS):
            for h in range(H):       # bkv itself is applied via v1 below
                # on DVE: the ACT queue must stay clear for the imminent
                # attention exps (kva here delayed them by ~10us)
                nc.vector.tensor_scalar_mul(
                    kva[:, tt, h, 0:DIM],
                    kvaccs[tt][:, h * DIM:(h + 1) * DIM], g[:, tt, h:h + 1])
            nc.vector.tensor_copy(kva[:, tt, :, DIM], g[:, tt, :])

        # ------------------------------ attention -----------------------------
        # Per ti, three phases so the PE queue never blocks on DVE/ACT:
        # (1) all 8 heads' score matmuls (+DVE bias adds, ACT exp),
        # (2) all probability-tile transposes (DVE copies trail),
        # (3) all attn@v accumulation matmuls (+ACT rz scale-out).
        attn_out = persist.tile([P, TS, D], F32)
        for ti in range(TS):
            nj = (ti + 1) * P
            j0 = ti * P - 99
            warmers(8 if ti < 2 else 12)
            ats = []
            for h in range(H):
                sp = psacc.tile([P, T], F32, name="sp", tag="acc")
                nc.tensor.matmul(
                    sp[:, 0:nj],
                    qT[hd(h):hd(h) + DIM, h // 2, ti * P:(ti + 1) * P],
                    keT[hd(h):hd(h) + DIM, h // 2, 0:nj],
                    start=True, stop=True)
                at = work.tile([P, T], BF16, name=f"at{h}", tag=f"at{h}",
                               bufs=1)
                nc.scalar.activation(out=at[:, 0:nj], in_=sp[:, 0:nj],
                                     func=AF.Exp, bias=0.0, scale=1.0)
                # banded bias and causal mask enter multiplicatively:
                # window holds exp(Db) in the band, 1.0 below, 0.0 above diag
                if ti == 0:
                    nc.vector.tensor_tensor(at[:, 0:P], at[:, 0:P],
                                            wins[0][:, h, 99:WIN], ALU.mult)
                else:
                    nc.vector.tensor_tensor(at[:, j0:j0 + WIN],
                                            at[:, j0:j0 + WIN],
                                            wins[ti][:, h, :], ALU.mult)
                ats.append(at)
            if ti >= 2:
                warmers(6)
            atTs = work.tile([P, H, (ti + 1) * P], BF16, name="atTs",
                             tag="atTs", bufs=2)
            for h in range(H):
                tp = psum.tile([P, ti + 1, P], BF16, name="attp", tag="pp")
                for js in range(ti + 1):
                    nc.tensor.transpose(tp[:, js, :],
                                        ats[h][:, js * P:(js + 1) * P],
                                        ident16)
                nc.vector.tensor_copy(
                    atTs[:, h, :].rearrange("p (a b) -> p a b", a=ti + 1), tp)
            if ti >= 2:
                warmers(6)
            for h in range(H):
                op = psacc.tile([P, DIM + 1], F32, name="avp", tag="acc")
                for js in range(ti + 1):
                    nc.tensor.matmul(op, atTs[:, h, js * P:(js + 1) * P],
                                     kva[:, js, h, :],
                                     start=(js == 0), stop=(js == ti))
                rz = work.tile([P, 1], F32, name="rz", tag="rz")
                nc.vector.reciprocal(rz, op[:, DIM:DIM + 1])
                nc.scalar.activation(
                    out=attn_out[:, ti, h * DIM:(h + 1) * DIM],
                    in_=op[:, 0:DIM],
                    func=AF.Identity, bias=0.0, scale=rz)

        # ------------------------ residual + block1 ---------------------------
        wh1 = wpool.tile([P, DS, HID], BF16, name="wh1", tag="w")
        nc.sync.dma_start(wh1, wh1p[:, :, :])
        v1 = persist.tile([P, TS, D], F32)
        warmers(12)
        for tt in range(TS):
            nc.vector.tensor_add(v1[:, tt, :], vals[:, tt, :],
                                 attn_out[:, tt, :])
            nc.vector.tensor_add(v1[:, tt, :], v1[:, tt, :],
                                 sm[:, SM_BKV:SM_BKV + D])
        ln1T = persist.tile([P, DS, T], BF16, name="ln1T", tag="lnT")
        layernorm_to_T(v1, sm[:, SM_LN1G:SM_LN1G + DS],
                       sm[:, SM_LN1B:SM_LN1B + DS], ln1T, "ln1")
        warmers(10)

        x1T = persist.tile([P, CS, T], BF16, name="x1T", tag="xT")
        for cs_ in range(CS):
            pp = psacc.tile([P, T], F32, name="h1pp", tag="acc")
            for es in range(DS):
                nc.tensor.matmul(pp, wh1[:, es, cs_ * P:(cs_ + 1) * P],
                                 ln1T[:, es, :],
                                 start=(es == 0), stop=(es == DS - 1))
            nc.vector.tensor_scalar(
                x1T[:, cs_, :], pp, sm[:, SM_BH1 + cs_:SM_BH1 + cs_ + 1], 0.0,
                op0=ALU.add, op1=ALU.max)

        wo1 = wpool.tile([P, CS, D], BF16, name="wo1", tag="w")
        nc.sync.dma_start(wo1, wo1p[:, :, :])
        o1accs = [psacc.tile([P, D], F32, name=f"o1a{t}", tag="acc")
                  for t in range(TS)]
        for cs_ in range(CS):      # cs_-outer: o1 starts on x1T chunk 0,
            for tt in range(TS):   # overlapping the rest of the h1 phase
                nc.tensor.matmul(o1accs[tt],
                                 x1T[:, cs_, tt * P:(tt + 1) * P],
                                 wo1[:, cs_, :],
                                 start=(cs_ == 0), stop=(cs_ == CS - 1))
        for tt in range(TS):
            fin = work.tile([P, D], F32, name="fin", tag="fin")
            nc.vector.tensor_add(fin, o1accs[tt], v1[:, tt, :])
            nc.vector.tensor_add(fin, fin, sm[:, SM_BO1:SM_BO1 + D])
            nc.sync.dma_start(out[:, tt, :], fin)

    if not nc.is_finalized():
        nc.finalize()
    return nc


def _pcol(v):
    """[D] -> [128, D//128] partition-major columns."""
    return np.ascontiguousarray(v.reshape(-1, P).T)


def _pmajor(w, rows_per_part):
    """[(s p), c] -> [128, s, c]."""
    s = rows_per_part
    return np.ascontiguousarray(
        w.reshape(s, P, w.shape[1]).transpose(1, 0, 2))


def build_in_maps(inputs):
    f32 = lambda x: np.asarray(x, dtype=np.float32)
    bf = lambda x: np.ascontiguousarray(x).astype(NPBF)

    rel101 = f32(inputs["rel_enc"])[:L + 1]                     # [101, D]
    wkr = f32(inputs["wkr"])
    wb1 = f32(inputs["wb1"])
    RW = (rel101 @ wkr).T                                       # [D, 101]
    rwd8 = 8.0 * (RW[:, 1:] - RW[:, 0:1])                       # [D, 100]
    rwdT = rwd8.reshape(DS, P, BW).transpose(1, 0, 2).reshape(P, DS * BW)
    E1 = rel101 @ wkr @ wb1                                     # [101, H]
    e1d = (E1[1:] - E1[0:1]).T                                  # [H, 100]

    smalls = np.zeros((P, NS), np.float32)
    smalls[:, SM_LN0G:SM_LN0G + DS] = _pcol(f32(inputs["ln0_g"]))
    smalls[:, SM_LN0B:SM_LN0B + DS] = _pcol(f32(inputs["ln0_b"]))
    smalls[:, SM_LN1G:SM_LN1G + DS] = _pcol(f32(inputs["ln1_g"]))
    smalls[:, SM_LN1B:SM_LN1B + DS] = _pcol(f32(inputs["ln1_b"]))
    smalls[:, SM_BH0:SM_BH0 + CS] = _pcol(f32(inputs["b_h0"]))
    smalls[:, SM_BH1:SM_BH1 + CS] = _pcol(f32(inputs["b_h1"]))
    smalls[:, SM_BQ:SM_BQ + DS] = _pcol(f32(inputs["bq"]))  # (acc+bq)*0.125
    smalls[:, SM_BKE:SM_BKE + DS] = _pcol(f32(inputs["bke"]))
    smalls[:, SM_ID32:SM_ID32 + P] = np.eye(P, dtype=np.float32)
    smalls[:, SM_BKV:SM_BKV + D] = np.tile(f32(inputs["bkv"]), (P, 1))
    smalls[:, SM_BO1:SM_BO1 + D] = np.tile(f32(inputs["b_o1"]), (P, 1))

    mask = np.asarray(inputs["values_mask"])
    maskbias = np.where(mask, 0.0, NEG).astype(np.float32)      # [B, T]

    smb_base = np.zeros((P, NSB), np.float32)
    smb_base[:, SB_ID16:SB_ID16 + P] = np.eye(P, dtype=np.float32)
    smb_base[0, SB_ONES:SB_ONES + P] = 1.0
    smb_base[:, SB_RWD:SB_RWD + DS * BW] = rwdT
    smb_base[0, SB_E1D:SB_E1D + H * BW] = e1d.reshape(-1)
    smb_base[:, SB_WB0:SB_WB0 + DS * H] = _pmajor(f32(inputs["wb0"]), DS
                                                  ).reshape(P, DS * H)
    fill = np.ones(WROW, np.float32)   # multiplicative window: exp(0)=1
    fill[WIN:] = 0.0                   # causal mask: exact zero factor
    smb_base[:, SB_FILL:SB_FILL + WROW] = fill[None, :]

    shared = {
        "smalls": smalls,
        "wh0p": bf(_pmajor(f32(inputs["w_h0"]), DS)),
        "wqp": bf(_pmajor(f32(inputs["wq"]), CS)),
        "wkep": bf(_pmajor(f32(inputs["wke"]), CS)),
        "wkvp": bf(_pmajor(f32(inputs["wkv"]), CS)),
        "wh1p": bf(_pmajor(f32(inputs["w_h1"]), DS)),
        "wo1p": bf(_pmajor(f32(inputs["w_o1"]), CS)),
    }

    vals = f32(inputs["values"])
    in_maps = []
    for b in range(B):
        m = dict(shared)
        m["values_b"] = np.ascontiguousarray(
            vals[b].reshape(TS, P, D).transpose(1, 0, 2))
        smb = smb_base.copy()
        smb[0, SB_MASK:SB_MASK + T] = maskbias[b]
        m["smallsb_b"] = bf(smb)
        in_maps.append(m)
    return in_maps


_NC_CACHE = None


def kernel(**inputs) -> np.ndarray:
    global _NC_CACHE
    if _NC_CACHE is None:
        _NC_CACHE = build_nc()
    nc = _NC_CACHE

    from concourse.bass_utils import run_bass_kernel_spmd

    in_maps = build_in_maps(inputs)
    res = run_bass_kernel_spmd(nc, in_maps, core_ids=list(range(B)))
    return np.stack(
        [res.results[b]["out_b"].transpose(1, 0, 2).reshape(T, D)
         for b in range(B)], axis=0)


if __name__ == "__main__":
    nc = build_nc()
    print("built ok")


# revision 47
# speedup vs baseline: 1.2895x; 1.0017x over previous
"""Trainium2 Bass kernel for EncoderWithPositionalAttentionLayer.

Sharding: data-parallel over batch B=8 across 8 NeuronCores (one batch
element per core).  The batch-independent relative-position algebra is
collapsed on the HOST (exact fp32 numpy):

  score[i,j] = q[i].ke[j]/8 + q[i].RW[:,idx] + E1[idx,h] + b0[j] (+consts)
  idx = clip(j-i,-100,100)+100; under the causal mask idx in [0,100].
  Terms constant along a score row (idx=0 tables, bb0/bb1, bkr terms)
  cancel in softmax.  What remains is a banded bias
     Db[i,t] = (q[i]/8).(8*RWD[:,t]) + E1D[t,h],  t = j-i+100 in [1,100]
  with RWD/E1D host-computed delta tables (vs idx=0).

On device, exp(Db) goes through a DRAM scratch with read-side skew:
rows of width 360 per (partition, head, itile) hold
[127 ones][100 exp(Db)][133 zeros]; one contiguous write, then a read
with partition-dependent offset (stride ROWS-1) yields a j-aligned
window that multiplies each probability tile after the score exp
(exp(s+w) = exp(s)*exp(w)), applying band bias AND causal mask at once.
bias0 (+mask bias) enters the same way as a per-column factor
g[j] = exp(b0[j]) folded into kv, whose appended 65th column (= g)
makes attn @ kv_aug produce the softmax denominator for free.

Everything on the main path is bf16 (matmul rate is 1 cycle/row, same
as fp32r, at any moving dim; PSUM accumulation stays fp32).  Weights
are host-prepacked partition-major so every weight DMA is 128
contiguous 16KB runs.
"""

import contextlib
import sys

sys.path.insert(0, "/opt/trn_rl_repo")

import numpy as np
import ml_dtypes

import concourse.bass as bass
from concourse import bacc
import concourse.mybir as mybir
import concourse.tile as tile

F32 = mybir.dt.float32
BF16 = mybir.dt.bfloat16
AF = mybir.ActivationFunctionType
ALU = mybir.AluOpType
NPBF = ml_dtypes.bfloat16

B, T, D, H, HID = 8, 512, 512, 8, 2048
DIM = D // H          # 64
L = 100
BW = L                # band width (t = 1..100)
EPS = 1e-3
P = 128
TS = T // P           # 4
DS = D // P           # 4
CS = HID // P         # 16
NEG = -60.0           # exp(-60) ~ 1e-26: exact-enough masking
WROW = 360            # scratch row: [127 zeros][100 Db][133 NEG]
NK = H * TS           # 32 scratch tiles (k = ti*H + h)
ROWS = NK * WROW      # per-partition scratch row block (11520)
WIN = 227             # j-aligned window width read back per tile

# smalls (fp32) column offsets
SM_LN0G, SM_LN0B, SM_LN1G, SM_LN1B = 0, 4, 8, 12
SM_BH0, SM_BH1 = 16, 32
SM_BQ, SM_BKE = 48, 52
SM_ID32 = 56
SM_BKV = SM_ID32 + 128          # 184
SM_BO1 = SM_BKV + 512           # 696
NS = SM_BO1 + 512               # 1208

# smallsb (bf16) column offsets
SB_ID16 = 0
SB_ONES = 128
SB_RWD = 256                    # [128, 4*100]
SB_E1D = SB_RWD + 400           # row 0: 8 heads x 100
SB_MASK = SB_E1D + 800          # row 0: maskbias [T]
SB_WB0 = SB_MASK + 512          # [128, 4*8]
SB_FILL = SB_WB0 + 32           # [128, 360] scratch row fill pattern
NSB = SB_FILL + WROW            # 2388


def build_nc():
    nc = bacc.Bacc()

    dp = nc.declare_dram_parameter
    values = dp("values_b", [P, TS, D], F32, isOutput=False)
    smalls = dp("smalls", [P, NS], F32, isOutput=False)
    smallsb = dp("smallsb_b", [P, NSB], BF16, isOutput=False)
    wh0p = dp("wh0p", [P, DS, HID], BF16, isOutput=False)
    wqp = dp("wqp", [P, CS, D], BF16, isOutput=False)
    wkep = dp("wkep", [P, CS, D], BF16, isOutput=False)
    wkvp = dp("wkvp", [P, CS, D], BF16, isOutput=False)
    wh1p = dp("wh1p", [P, DS, HID], BF16, isOutput=False)
    wo1p = dp("wo1p", [P, CS, D], BF16, isOutput=False)
    out = dp("out_b", [P, TS, D], F32, isOutput=True)

    with tile.TileContext(nc) as tc, contextlib.ExitStack() as ctx:
        persist = ctx.enter_context(tc.tile_pool(name="persist", bufs=1))
        wpool = ctx.enter_context(tc.tile_pool(name="wpool", bufs=3))
        work = ctx.enter_context(tc.tile_pool(name="work", bufs=3))
        psum = ctx.enter_context(tc.tile_pool(name="psum", bufs=3, space="PSUM"))
        psacc = ctx.enter_context(tc.tile_pool(name="psacc", bufs=4, space="PSUM"))
        pwarm = ctx.enter_context(tc.tile_pool(name="pwarm", bufs=1, space="PSUM"))
        dram = ctx.enter_context(tc.tile_pool(name="dram", bufs=1, space="DRAM"))

        # ---------------- input DMAs ----------------------------------------
        # Everything early rides the sync ring: the scalar (ACT) engine
        # spends its first ~10us loading activation tables, which would
        # delay DMAs issued from it.  The scalar ring only carries the
        # mid-kernel scratch round-trip.
        # tiny boot DMA (identity + ones, 64KB) so PE warmers start ~1.5us in
        smbb = persist.tile([P, 256], BF16)
        nc.sync.dma_start(smbb, smallsb[:, 0:256])
        vals = persist.tile([P, TS, D], F32)
        nc.sync.dma_start(vals, values[:, :, :])
        sm = persist.tile([P, NS], F32)
        nc.sync.dma_start(sm, smalls[:, :])
        smb = persist.tile([P, NSB], BF16)
        nc.sync.dma_start(smb, smallsb[:, :])

        # ---------------- weight DMAs (sync ring, use order) -----------------
        wh0 = wpool.tile([P, DS, HID], BF16, name="wh0", tag="w")
        nc.sync.dma_start(wh0, wh0p[:, :, :])
        wq = wpool.tile([P, CS, D], BF16, name="wq", tag="w")
        nc.sync.dma_start(wq, wqp[:, :, :])
        wke = wpool.tile([P, CS, D], BF16, name="wke", tag="w")
        nc.sync.dma_start(wke, wkep[:, :, :])

        ident32 = sm[:, SM_ID32:SM_ID32 + 128]
        ident16 = smbb[:, 0:128]
        onesb = smbb[0:1, 128:256]
        rwdT = smb[:, SB_RWD:SB_RWD + 400].rearrange("p (s t) -> p s t", s=DS)
        wb0_sb = smb[:, SB_WB0:SB_WB0 + 32].rearrange("p (s h) -> p s h", s=DS)
        fill = smb[:, SB_FILL:SB_FILL + WROW]
        eps_sb = persist.tile([P, 1], F32)
        nc.vector.memset(eps_sb, EPS)

        # PE warmers: dependency-free matmuls that keep the HAM clock at
        # 8/8 while real matmul operands are still in flight.  One call is
        # a single accumulation group so the matmuls stream back-to-back
        # (per-matmul start/stop would serialize on the bank drain).
        def warmers(n, wide=False):
            # wide=True streams N=512 from smb (only once smb has landed)
            rhs = smb[:, 0:512] if wide else smbb[:, 0:256]
            wps = pwarm.tile([P, 512], F32, name="warm", tag="warm")
            for i in range(n):
                nc.tensor.matmul(wps[:, 0:rhs.shape[-1]], ident16, rhs,
                                 start=(i == 0), stop=(i == n - 1))

        warmers(50)

        # scratch fill+band SBUF image: [P, NK, WROW] bf16 (23KB/partition).
        # The fill copies are emitted later (just before the Db section) so
        # they queue behind the LN0 work on DVE instead of ahead of it.
        fb = persist.tile([P, NK, WROW], BF16)

        # --------------------------- LN helper --------------------------------
        def layernorm_to_T(x_tiles, gcol, bcol, lnT_out, name):
            for tt in range(TS):
                xt = x_tiles[:, tt, :]
                stats = work.tile([P, 6], F32, name=f"{name}st{tt}", tag="lnst")
                nc.vector.bn_stats(out=stats, in_=xt)
                mv = work.tile([P, 2], F32, name=f"{name}mv{tt}", tag="lnmv")
                nc.vector.bn_aggr(out=mv, in_=stats)
                rstd = work.tile([P, 1], F32, name=f"{name}rs{tt}", tag="lnrs")
                nc.scalar.activation(out=rstd, in_=mv[:, 1:2], func=AF.Sqrt,
                                     bias=eps_sb, scale=1.0)
                nc.vector.reciprocal(rstd, rstd)
                xn = work.tile([P, D], F32, name=f"{name}xn{tt}", tag="lnxn")
                nc.vector.tensor_scalar(xn, xt, mv[:, 0:1], rstd,
                                        op0=ALU.subtract, op1=ALU.mult)
                for es in range(DS):
                    tp = psum.tile([P, P], F32, name=f"{name}tp", tag="pp")
                    nc.tensor.transpose(tp, xn[:, es * P:(es + 1) * P], ident32)
                    nc.vector.tensor_scalar(
                        lnT_out[:, es, tt * P:(tt + 1) * P], tp,
                        gcol[:, es:es + 1], bcol[:, es:es + 1],
                        op0=ALU.mult, op1=ALU.add)

        # ------------------------- LN0 + block0 ------------------------------
        ln0T = persist.tile([P, DS, T], BF16, name="ln0T", tag="lnT")
        layernorm_to_T(vals, sm[:, SM_LN0G:SM_LN0G + DS],
                       sm[:, SM_LN0B:SM_LN0B + DS], ln0T, "ln0")

        xT = persist.tile([P, CS, T], BF16, name="xT", tag="xT")
        for cs_ in range(CS):
            pp = psacc.tile([P, T], F32, name="h0pp", tag="acc")
            for es in range(DS):
                nc.tensor.matmul(pp, wh0[:, es, cs_ * P:(cs_ + 1) * P],
                                 ln0T[:, es, :],
                                 start=(es == 0), stop=(es == DS - 1))
            nc.vector.tensor_scalar(      # fused bias + relu on DVE
                xT[:, cs_, :], pp, sm[:, SM_BH0 + cs_:SM_BH0 + cs_ + 1], 0.0,
                op0=ALU.add, op1=ALU.max)

        # --------------------------- projections -----------------------------
        def project_T(w_sb, dest, boff, scale):
            """dest [128(d), DS, T] (bf16) = scale*((x @ w).T + b)."""
            accs = [psacc.tile([P, T], F32, name=f"pa{d}", tag="acc")
                    for d in range(DS)]
            for cs_ in range(CS):
                for dsub in range(DS):
                    nc.tensor.matmul(
                        accs[dsub], w_sb[:, cs_, dsub * P:(dsub + 1) * P],
                        xT[:, cs_, :],
                        start=(cs_ == 0), stop=(cs_ == CS - 1))
            for dsub in range(DS):
                nc.vector.tensor_scalar(   # (acc + b) * scale on DVE
                    dest[:, dsub, :], accs[dsub],
                    sm[:, boff + dsub:boff + dsub + 1], scale,
                    op0=ALU.add, op1=ALU.mult)

        qT = persist.tile([P, DS, T], BF16)      # holds q/8 transposed
        project_T(wq, qT, SM_BQ, 0.125)          # bias pre-scaled on host

        # ---------------- Db tiles (interleaved with keT) ---------------------
        # fill margins: row layout [127 zeros][100 Db][133 NEG], two
        # broadcast copies over all NK rows
        nc.vector.tensor_copy(
            fb[:, :, 0:127],
            fill[:, 0:127].rearrange("p (a w) -> p a w", a=1)
                .to_broadcast((P, NK, 127)))
        nc.vector.tensor_copy(
            fb[:, :, WIN:WROW],
            fill[:, WIN:WROW].rearrange("p (a w) -> p a w", a=1)
                .to_broadcast((P, NK, WROW - WIN)))
        hd = lambda h: (h % 2) * DIM

        def db_group(ti, hh):
            """Db for heads 4*hh..4*hh+3 of row-tile ti into one PSUM bank."""
            dbp = psum.tile([P, 4, BW], F32, name="dbp", tag="pp")
            for i4 in range(4):
                h = hh * 4 + i4
                nc.tensor.matmul(
                    dbp[:, i4, :],
                    qT[hd(h):hd(h) + DIM, h // 2, ti * P:(ti + 1) * P],
                    rwdT[hd(h):hd(h) + DIM, h // 2, :],
                    start=True, stop=False)
                nc.tensor.matmul(
                    dbp[:, i4, :], onesb,
                    smb[0:1, SB_E1D + h * BW:SB_E1D + (h + 1) * BW],
                    start=False, stop=True)
            # store exp(Db): the window is applied multiplicatively after
            # the score exp (exp(s+w) = exp(s)*exp(w)); fill is 1.0 / 0.0
            nc.scalar.activation(
                out=fb[:, ti * H + hh * 4:ti * H + hh * 4 + 4, 127:227],
                in_=dbp, func=AF.Exp, bias=0.0, scale=1.0)

        # keT projection with Db groups interleaved so the PE array duty
        # cycle stays high (Db matmuls alone are LDW-dominated)
        keT = persist.tile([P, DS, T], BF16)
        keaccs = [psacc.tile([P, T], F32, name=f"kea{d}", tag="acc")
                  for d in range(DS)]
        for cs_ in range(CS):
            for dsub in range(DS):
                nc.tensor.matmul(
                    keaccs[dsub], wke[:, cs_, dsub * P:(dsub + 1) * P],
                    xT[:, cs_, :],
                    start=(cs_ == 0), stop=(cs_ == CS - 1))
            if cs_ % 2 == 1:
                db_group((cs_ - 1) // 4, ((cs_ - 1) // 2) % 2)
        for dsub in range(DS):
            nc.vector.tensor_scalar(
                keT[:, dsub, :], keaccs[dsub],
                sm[:, SM_BKE + dsub:SM_BKE + dsub + 1], 1.0,
                op0=ALU.add, op1=ALU.mult)

        scr = dram.tile([P * ROWS], BF16, name="scr")
        nc.scalar.dma_start(
            bass.AP(tensor=scr.tensor, offset=scr.offset,
                    ap=[[ROWS, P], [WROW, NK], [1, WROW]]),
            fb)
        # skewed window read: win[p, k, w] = scr[p*ROWS + k*WROW + 127 + w - p]
        wins = []
        for ti in range(TS):
            wr = work.tile([P, H, WIN], BF16, name=f"win{ti}", tag=f"win{ti}",
                           bufs=1)
            nc.scalar.dma_start(
                wr,
                bass.AP(tensor=scr.tensor,
                        offset=scr.offset + ti * H * WROW + 127,
                        ap=[[ROWS - 1, P], [WROW, H], [1, WIN]]))
            wins.append(wr)

        # bias0 (+maskbias) enters the softmax as a per-column factor
        # g[j] = exp(b0[j] + maskbias[j]): fold it into kv, with a 65th
        # column equal to g so attn@kv_aug yields the softmax denominator.
        b0p = psum.tile([H, T], F32, name="b0p", tag="pp")
        for c in range(DS):
            nc.tensor.matmul(b0p, wb0_sb[:, c, :], keT[:, c, :],
                             start=(c == 0), stop=False)
        nc.tensor.matmul(b0p, onesb[:, 0:H], smb[0:1, SB_MASK:SB_MASK + T],
                         start=False, stop=True)
        b0m = work.tile([H, T], F32, name="b0m", tag="b0m", bufs=1)
        nc.vector.tensor_copy(b0m, b0p)
        g = persist.tile([P, TS, H], F32)
        for tt in range(TS):
            gp = psum.tile([P, H], F32, name="gp", tag="pp")
            nc.tensor.matmul(gp, b0m[:, tt * P:(tt + 1) * P],
                             ident32[0:H, 0:H], start=True, stop=True)
            nc.scalar.activation(out=g[:, tt, :], in_=gp, func=AF.Exp,
                                 bias=0.0, scale=1.0)

        wkv = wpool.tile([P, CS, D], BF16, name="wkv", tag="w")
        nc.sync.dma_start(wkv, wkvp[:, :, :])
        kva = persist.tile([P, TS, H, DIM + 1], BF16)
        kvaccs = [psacc.tile([P, D], F32, name=f"kva{t}", tag="acc")
                  for t in range(TS)]
        for cs_ in range(CS):
            for tt in range(TS):
                nc.tensor.matmul(kvaccs[tt],
                                 xT[:, cs_, tt * P:(tt + 1) * P],
                                 wkv[:, cs_, :],
                                 start=(cs_ == 0), stop=(cs_ == CS - 1))
        for tt in range(TS):
            for h in range(H):       # bkv itself is applied via v1 below
                # on DVE: the ACT queue must stay clear for the imminent
                # attention exps (kva here delayed them by ~10us)
                nc.vector.tensor_scalar_mul(
                    kva[:, tt, h, 0:DIM],
                    kvaccs[tt][:, h * DIM:(h + 1) * DIM], g[:, tt, h:h + 1])
            nc.vector.tensor_copy(kva[:, tt, :, DIM], g[:, tt, :])

        # ------------------------------ attention -----------------------------
        # Per ti, three phases so the PE queue never blocks on DVE/ACT:
        # (1) all 8 heads' score matmuls (+DVE bias adds, ACT exp),
        # (2) all probability-tile transposes (DVE copies trail),
        # (3) all attn@v accumulation matmuls (+ACT rz scale-out).
        attn_out = persist.tile([P, TS, D], F32)
        for ti in range(TS):
            nj = (ti + 1) * P
            j0 = ti * P - 99
            warmers(8 if ti < 2 else 12, wide=True)
            ats = []
            for h in range(H):
                sp = psacc.tile([P, T], F32, name="sp", tag="acc")
                nc.tensor.matmul(
                    sp[:, 0:nj],
                    qT[hd(h):hd(h) + DIM, h // 2, ti * P:(ti + 1) * P],
                    keT[hd(h):hd(h) + DIM, h // 2, 0:nj],
                    start=True, stop=True)
                at = work.tile([P, T], BF16, name=f"at{h}", tag=f"at{h}",
                               bufs=1)
                nc.scalar.activation(out=at[:, 0:nj], in_=sp[:, 0:nj],
                                     func=AF.Exp, bias=0.0, scale=1.0)
                # banded bias and causal mask enter multiplicatively:
                # window holds exp(Db) in the band, 1.0 below, 0.0 above diag
                if ti == 0:
                    nc.vector.tensor_tensor(at[:, 0:P], at[:, 0:P],
                                            wins[0][:, h, 99:WIN], ALU.mult)
                else:
                    nc.vector.tensor_tensor(at[:, j0:j0 + WIN],
                                            at[:, j0:j0 + WIN],
                                            wins[ti][:, h, :], ALU.mult)
                ats.append(at)
            warmers(8, wide=True)
            atTs = work.tile([P, H, (ti + 1) * P], BF16, name="atTs",
                             tag="atTs", bufs=2)
            for h in range(H):
                tp = psum.tile([P, ti + 1, P], BF16, name="attp", tag="pp")
                for js in range(ti + 1):
                    nc.tensor.transpose(tp[:, js, :],
                                        ats[h][:, js * P:(js + 1) * P],
                                        ident16)
                nc.vector.tensor_copy(
                    atTs[:, h, :].rearrange("p (a b) -> p a b", a=ti + 1), tp)
            warmers(8, wide=True)
            for h in range(H):
                op = psacc.tile([P, DIM + 1], F32, name="avp", tag="acc")
                for js in range(ti + 1):
                    nc.tensor.matmul(op, atTs[:, h, js * P:(js + 1) * P],
                                     kva[:, js, h, :],
                                     start=(js == 0), stop=(js == ti))
                rz = work.tile([P, 1], F32, name="rz", tag="rz")
                nc.vector.reciprocal(rz, op[:, DIM:DIM + 1])
                nc.scalar.activation(
                    out=attn_out[:, ti, h * DIM:(h + 1) * DIM],
                    in_=op[:, 0:DIM],
                    func=AF.Identity, bias=0.0, scale=rz)

        # ------------------------ residual + block1 ---------------------------
        wh1 = wpool.tile([P, DS, HID], BF16, name="wh1", tag="w")
        nc.sync.dma_start(wh1, wh1p[:, :, :])
        v1 = persist.tile([P, TS, D], F32)
        warmers(12, wide=True)
        for tt in range(TS):
            nc.vector.tensor_add(v1[:, tt, :], vals[:, tt, :],
                                 attn_out[:, tt, :])
            nc.vector.tensor_add(v1[:, tt, :], v1[:, tt, :],
                                 sm[:, SM_BKV:SM_BKV + D])
        ln1T = persist.tile([P, DS, T], BF16, name="ln1T", tag="lnT")
        layernorm_to_T(v1, sm[:, SM_LN1G:SM_LN1G + DS],
                       sm[:, SM_LN1B:SM_LN1B + DS], ln1T, "ln1")
        warmers(10, wide=True)

        x1T = persist.tile([P, CS, T], BF16, name="x1T", tag="xT")
        for cs_ in range(CS):
            pp = psacc.tile([P, T], F32, name="h1pp", tag="acc")
            for es in range(DS):
                nc.tensor.matmul(pp, wh1[:, es, cs_ * P:(cs_ + 1) * P],
                                 ln1T[:, es, :],
                                 start=(es == 0), stop=(es == DS - 1))
            nc.vector.tensor_scalar(
                x1T[:, cs_, :], pp, sm[:, SM_BH1 + cs_:SM_BH1 + cs_ + 1], 0.0,
                op0=ALU.add, op1=ALU.max)

        wo1 = wpool.tile([P, CS, D], BF16, name="wo1", tag="w")
        nc.sync.dma_start(wo1, wo1p[:, :, :])
        o1accs = [psacc.tile([P, D], F32, name=f"o1a{t}", tag="acc")
                  for t in range(TS)]
        for cs_ in range(CS):      # cs_-outer: o1 starts on x1T chunk 0,
            for tt in range(TS):   # overlapping the rest of the h1 phase
                nc.tensor.matmul(o1accs[tt],
                                 x1T[:, cs_, tt * P:(tt + 1) * P],
                                 wo1[:, cs_, :],
                                 start=(cs_ == 0), stop=(cs_ == CS - 1))
        for tt in range(TS):
            fin = work.tile([P, D], F32, name="fin", tag="fin")
            nc.vector.tensor_add(fin, o1accs[tt], v1[:, tt, :])
            nc.vector.tensor_add(fin, fin, sm[:, SM_BO1:SM_BO1 + D])
            nc.sync.dma_start(out[:, tt, :], fin)

    if not nc.is_finalized():
        nc.finalize()
    return nc


def _pcol(v):
    """[D] -> [128, D//128] partition-major columns."""
    return np.ascontiguousarray(v.reshape(-1, P).T)


def _pmajor(w, rows_per_part):
    """[(s p), c] -> [128, s, c]."""
    s = rows_per_part
    return np.ascontiguousarray(
        w.reshape(s, P, w.shape[1]).transpose(1, 0, 2))


def build_in_maps(inputs):
    f32 = lambda x: np.asarray(x, dtype=np.float32)
    bf = lambda x: np.ascontiguousarray(x).astype(NPBF)

    rel101 = f32(inputs["rel_enc"])[:L + 1]                     # [101, D]
    wkr = f32(inputs["wkr"])
    wb1 = f32(inputs["wb1"])
    RW = (rel101 @ wkr).T                                       # [D, 101]
    rwd8 = 8.0 * (RW[:, 1:] - RW[:, 0:1])                       # [D, 100]
    rwdT = rwd8.reshape(DS, P, BW).transpose(1, 0, 2).reshape(P, DS * BW)
    E1 = rel101 @ wkr @ wb1                                     # [101, H]
    e1d = (E1[1:] - E1[0:1]).T                                  # [H, 100]

    smalls = np.zeros((P, NS), np.float32)
    smalls[:, SM_LN0G:SM_LN0G + DS] = _pcol(f32(inputs["ln0_g"]))
    smalls[:, SM_LN0B:SM_LN0B + DS] = _pcol(f32(inputs["ln0_b"]))
    smalls[:, SM_LN1G:SM_LN1G + DS] = _pcol(f32(inputs["ln1_g"]))
    smalls[:, SM_LN1B:SM_LN1B + DS] = _pcol(f32(inputs["ln1_b"]))
    smalls[:, SM_BH0:SM_BH0 + CS] = _pcol(f32(inputs["b_h0"]))
    smalls[:, SM_BH1:SM_BH1 + CS] = _pcol(f32(inputs["b_h1"]))
    smalls[:, SM_BQ:SM_BQ + DS] = _pcol(f32(inputs["bq"]))  # (acc+bq)*0.125
    smalls[:, SM_BKE:SM_BKE + DS] = _pcol(f32(inputs["bke"]))
    smalls[:, SM_ID32:SM_ID32 + P] = np.eye(P, dtype=np.float32)
    smalls[:, SM_BKV:SM_BKV + D] = np.tile(f32(inputs["bkv"]), (P, 1))
    smalls[:, SM_BO1:SM_BO1 + D] = np.tile(f32(inputs["b_o1"]), (P, 1))

    mask = np.asarray(inputs["values_mask"])
    maskbias = np.where(mask, 0.0, NEG).astype(np.float32)      # [B, T]

    smb_base = np.zeros((P, NSB), np.float32)
    smb_base[:, SB_ID16:SB_ID16 + P] = np.eye(P, dtype=np.float32)
    smb_base[0, SB_ONES:SB_ONES + P] = 1.0
    smb_base[:, SB_RWD:SB_RWD + DS * BW] = rwdT
    smb_base[0, SB_E1D:SB_E1D + H * BW] = e1d.reshape(-1)
    smb_base[:, SB_WB0:SB_WB0 + DS * H] = _pmajor(f32(inputs["wb0"]), DS
                                                  ).reshape(P, DS * H)
    fill = np.ones(WROW, np.float32)   # multiplicative window: exp(0)=1
    fill[WIN:] = 0.0                   # causal mask: exact zero factor
    smb_base[:, SB_FILL:SB_FILL + WROW] = fill[None, :]

    shared = {
        "smalls": smalls,
        "wh0p": bf(_pmajor(f32(inputs["w_h0"]), DS)),
        "wqp": bf(_pmajor(f32(inputs["wq"]), CS)),
        "wkep": bf(_pmajor(f32(inputs["wke"]), CS)),
        "wkvp": bf(_pmajor(f32(inputs["wkv"]), CS)),
        "wh1p": bf(_pmajor(f32(inputs["w_h1"]), DS)),
        "wo1p": bf(_pmajor(f32(inputs["w_o1"]), CS)),
    }

    vals = f32(inputs["values"])
    in_maps = []
    for b in range(B):
        m = dict(shared)
        m["values_b"] = np.ascontiguousarray(
            vals[b].reshape(TS, P, D).transpose(1, 0, 2))
        smb = smb_base.copy()
        smb[0, SB_MASK:SB_MASK + T] = maskbias[b]
        m["smallsb_b"] = bf(smb)
        in_maps.append(m)
    return in_maps


_NC_CACHE = None


def kernel(**inputs) -> np.ndarray:
    global _NC_CACHE
    if _NC_CACHE is None:
        _NC_CACHE = build_nc()
    nc = _NC_CACHE

    from concourse.bass_utils import run_bass_kernel_spmd

    in_maps = build_in_maps(inputs)
    res = run_bass_kernel_spmd(nc, in_maps, core_ids=list(range(B)))
    return np.stack(
        [res.results[b]["out_b"].transpose(1, 0, 2).reshape(T, D)
         for b in range(B)], axis=0)


if __name__ == "__main__":
    nc = build_nc()
    print("built ok")
